# revision 3
# baseline (speedup 1.0000x reference)
# Trainium2 Bass kernel for nn_DecoderBlock — transposed-activation design.
#
# Sharding: data-parallel over batch — 16 elems / 8 cores = 2 per core.
#
# All activations live in SBUF in TRANSPOSED layout xT [D(part, 8 chunks), S]
# for the whole kernel; natural layout is never materialized on device (the
# host transposes the final output back, which is free).
#
# Per head h:
#   QT = Wq[h].T @ xT            (lhsT = Wq chunks, rhs = xT)      [dq, S]
#   expQT = exp((QT + mask)/sc)  -> qs = colsum via ones-matmul    [1, S]
#   softQT = expQT * bcast(1/qs) (GPSIMD partition_broadcast)
#   K/V natural per sm-tile: lhsT = xT s-chunk, rhs = Wk/Wv packed [s, 8*dq]
#   expK, V' = V / rowsum(expK); A[h] += expK[h].T @ V'[h]         [dq, dq]
#   BmT[h] = A[h].T @ softQT                                       [dq, S]
# Output (torch .view(b,w,h*d) quirk folded into a strided AP):
#   outT[cc] = sum_j WoT[j,cc].T @ BmTpack[:, w', j]   (w' = 128h+q, s = 8q+j)
# Residual + LayerNorm in transposed layout: stats over the partition axis
# via ones-matmuls, per-column scale/bias replicated with partition_broadcast.
# LFFN fully transposed: h1T = E1 @ yT, h2T = D1 @ h1T, silu, E2, D2.
import numpy as np
import ml_dtypes

import concourse.bacc as bacc
import concourse.mybir as mybir
import concourse.tile as tile
from concourse.bass_utils import run_bass_kernel_spmd

H, D, DQ, BNK, HID = 8, 1024, 128, 512, 1024
B, S_T, S_M = 16, 1024, 2048
SCALE = DQ ** 0.25
EPS = 1e-5
NEG = -200.0
N_CORES = 8
BPC = B // N_CORES

f32 = mybir.dt.float32
bf16 = mybir.dt.bfloat16
AF = mybir.ActivationFunctionType
ALU = mybir.AluOpType
bf = ml_dtypes.bfloat16


def _ln_tail(nc, sb, mean_ps, msq_ps, eps_t):
    """Column stats [1,S] -> broadcast scale/bias tiles [128,S] f32."""
    mu = sb.tile([1, S_T], f32, tag="ln_t", bufs=4, name="mu")
    nc.scalar.activation(mu[:], mean_ps[:], AF.Identity, scale=1.0 / D)
    ex2 = sb.tile([1, S_T], f32, tag="ln_t", bufs=4, name="ex2")
    nc.scalar.activation(ex2[:], msq_ps[:], AF.Identity, scale=1.0 / D)
    var = sb.tile([1, S_T], f32, tag="ln_t", bufs=4, name="var")
    nc.vector.tensor_tensor(out=var[:], in0=mu[:], in1=mu[:], op=ALU.mult)
    nc.vector.tensor_tensor(out=var[:], in0=ex2[:], in1=var[:], op=ALU.subtract)
    sd = sb.tile([1, S_T], f32, tag="ln_t", bufs=4, name="sd")
    nc.scalar.activation(sd[:], var[:], AF.Sqrt, bias=eps_t[:])
    rstd = sb.tile([1, S_T], f32, tag="ln_r", bufs=2, name="rstd")
    nc.vector.reciprocal(rstd[:], sd[:])
    nmu = sb.tile([1, S_T], f32, tag="ln_r", bufs=2, name="nmu")
    nc.vector.scalar_tensor_tensor(
        out=nmu[:], in0=mu[:], scalar=-1.0, in1=rstd[:],
        op0=ALU.mult, op1=ALU.mult)
    scaleb = sb.tile([128, S_T], f32, tag="ln_scaleb", bufs=1)
    nc.gpsimd.partition_broadcast(scaleb[:], rstd[:])
    biasb = sb.tile([128, S_T], f32, tag="ln_biasb", bufs=1)
    nc.gpsimd.partition_broadcast(biasb[:], nmu[:])
    return scaleb, biasb


def _kv_stage(nc, tc, sb, wp, pa, yprev, memsm_dram, wk, wv, n_kv):
    """K/V natural projections + A accumulation for one batch elem.
    Returns a_sb [128, 8, DQ] bf16 (A per head). A matmuls run one sm-tile
    behind the projections so the PE never waits on the evac chain."""
    a_sb = sb.tile([128, 8, DQ], bf16, tag="a_sb", bufs=2)
    a_acc = sb.tile([128, 1024], f32, tag="a_acc", bufs=2)
    pend = []

    # NOTE: psum accumulation groups must not interleave within one bank, so
    # each sm's A-partial is a single-shot matmul set, accumulated on the DVE.
    def emit_a(sm, ek, ev):
        apart = pa.tile([128, 1024], f32, tag="apart", bufs=1)
        for h in range(H):
            nc.tensor.matmul(apart[:, DQ * h:DQ * (h + 1)],
                             ek[:, h, :], ev[:, h, :])
        if sm == 0:
            nc.vector.tensor_copy(a_acc[:], apart[:])
        else:
            nc.vector.tensor_tensor(out=a_acc[:], in0=a_acc[:], in1=apart[:],
                                    op=ALU.add)

    for sm in range(n_kv):
        if memsm_dram is None:
            def lhsT(k, sm=sm):
                return yprev[:, k, DQ * sm:DQ * (sm + 1)]
        else:
            mt = wp.tile([128, 8, DQ], bf16, tag="memsm", bufs=4)
            nc.sync.dma_start(mt[:], memsm_dram[sm])
            def lhsT(k, mt=mt):
                return mt[:, k, :]
        klo = pa.tile([128, 512], f32, tag="kv", bufs=6, name="klo")
        khi = pa.tile([128, 512], f32, tag="kv", bufs=6, name="khi")
        vlo = pa.tile([128, 512], f32, tag="kv", bufs=6, name="vlo")
        vhi = pa.tile([128, 512], f32, tag="kv", bufs=6, name="vhi")
        for k in range(8):
            lt = lhsT(k)
            nc.tensor.matmul(klo[:], lt, wk[:, k, 0:4, :], start=(k == 0), stop=(k == 7))
            nc.tensor.matmul(khi[:], lt, wk[:, k, 4:8, :], start=(k == 0), stop=(k == 7))
            nc.tensor.matmul(vlo[:], lt, wv[:, k, 0:4, :], start=(k == 0), stop=(k == 7))
            nc.tensor.matmul(vhi[:], lt, wv[:, k, 4:8, :], start=(k == 0), stop=(k == 7))
        expk = wp.tile([128, 8, DQ], bf16, tag="expk", bufs=3)
        nc.scalar.activation(expk[:, 0:4, :], klo[:], AF.Exp, scale=1.0 / SCALE)
        nc.scalar.activation(expk[:, 4:8, :], khi[:], AF.Exp, scale=1.0 / SCALE)
        krs = wp.tile([128, 8], f32, tag="krs", bufs=2)
        nc.vector.tensor_reduce(out=krs[:], in_=expk[:],
                                axis=mybir.AxisListType.X, op=ALU.add)
        krr = wp.tile([128, 8], f32, tag="krr", bufs=2)
        nc.vector.reciprocal(krr[:], krs[:])
        expv = wp.tile([128, 8, DQ], bf16, tag="expv", bufs=3)
        nc.vector.tensor_tensor(
            out=expv[:, 0:4, :], in0=vlo[:].rearrange("p (h q) -> p h q", h=4),
            in1=krr[:, 0:4].unsqueeze(2).broadcast_to([128, 4, DQ]), op=ALU.mult)
        nc.vector.tensor_tensor(
            out=expv[:, 4:8, :], in0=vhi[:].rearrange("p (h q) -> p h q", h=4),
            in1=krr[:, 4:8].unsqueeze(2).broadcast_to([128, 4, DQ]), op=ALU.mult)
        pend.append((sm, expk, expv))
        if len(pend) > 1:
            emit_a(*pend.pop(0))
    emit_a(*pend.pop(0))
    nc.vector.tensor_copy(a_sb[:].rearrange("p h q -> p (h q)"), a_acc[:])
    return a_sb


def _q_stage(nc, tc, sb, wp, pq, yprev, a_sb, wq, masked, maskt, ones_bf):
    """Q proj + softmax + BmT for one batch elem -> bmtp [128, 8, S_T].
    Pass-structured so the PE stream never waits on the softmax chain."""
    bmtp = sb.tile([128, 8, S_T], bf16, tag="bmtp", bufs=2)
    expqs = []
    for h in range(H):
        qt = pq.tile([128, S_T], f32, tag="qt", bufs=3)
        for k in range(8):
            nc.tensor.matmul(qt[:, 0:512], wq[:, h, k, :], yprev[:, k, 0:512],
                             start=(k == 0), stop=(k == 7))
            nc.tensor.matmul(qt[:, 512:1024], wq[:, h, k, :], yprev[:, k, 512:1024],
                             start=(k == 0), stop=(k == 7))
        if masked:
            nc.vector.tensor_tensor(out=qt[:, 0:DQ], in0=qt[:, 0:DQ],
                                    in1=maskt[:], op=ALU.add)
        expq = wp.tile([128, S_T], bf16, tag="expq", bufs=8, name=f"expq{h}")
        nc.scalar.activation(expq[:], qt[:], AF.Exp, scale=1.0 / SCALE)
        expqs.append(expq)
    for h in range(H):
        qs = pq.tile([1, S_T], f32, tag="qs", bufs=1)
        nc.tensor.matmul(qs[:, 0:512], ones_bf[:], expqs[h][:, 0:512])
        nc.tensor.matmul(qs[:, 512:1024], ones_bf[:], expqs[h][:, 512:1024])
        qsr = wp.tile([1, S_T], f32, tag="qsr", bufs=2)
        nc.vector.reciprocal(qsr[:], qs[:])
        qsb = wp.tile([128, S_T], f32, tag="qsb", bufs=2)
        nc.gpsimd.partition_broadcast(qsb[:], qsr[:])
        nc.vector.tensor_tensor(out=expqs[h][:], in0=expqs[h][:], in1=qsb[:],
                                op=ALU.mult)
    for h in range(H):
        bmt = pq.tile([128, S_T], f32, tag="qt", bufs=3)
        nc.tensor.matmul(bmt[:, 0:512], a_sb[:, h, :], expqs[h][:, 0:512])
        nc.tensor.matmul(bmt[:, 512:1024], a_sb[:, h, :], expqs[h][:, 512:1024])
        nc.scalar.activation(bmtp[:, h, :], bmt[:], AF.Identity)
    return bmtp


def _out_stage(nc, tc, sb, wp, po, yprev, ynext, bmtp, wo, ones_bf, eps_t,
               out_f32=False, gb=None, gbi=0):
    """Wo matmul (transposed out) + residual + LN for one batch elem.
    Stats matmuls run one cc-tile behind so the PE never waits."""
    bmv = bmtp[:].rearrange("p h (m e) -> p (h m) e", e=8)
    rsd = wp.tile([128, 8, S_T], bf16, tag="rsd", bufs=1)
    mean_ps = po.tile([1, S_T], f32, tag="mean", bufs=1)
    msq_ps = po.tile([1, S_T], f32, tag="msq", bufs=1)
    pend = []

    def emit_stats(cc, rsq):
        nc.tensor.matmul(mean_ps[:, 0:512], ones_bf[:], rsd[:, cc, 0:512],
                         start=(cc == 0), stop=(cc == 7))
        nc.tensor.matmul(mean_ps[:, 512:1024], ones_bf[:], rsd[:, cc, 512:1024],
                         start=(cc == 0), stop=(cc == 7))
        nc.tensor.matmul(msq_ps[:, 0:512], ones_bf[:], rsq[:, 0:512],
                         start=(cc == 0), stop=(cc == 7))
        nc.tensor.matmul(msq_ps[:, 512:1024], ones_bf[:], rsq[:, 512:1024],
                         start=(cc == 0), stop=(cc == 7))

    for cc in range(8):
        ot = po.tile([128, S_T], f32, tag="ot", bufs=2)
        for j in range(8):
            nc.tensor.matmul(ot[:, 0:512], wo[:, j, cc, :], bmv[:, 0:512, j],
                             start=(j == 0), stop=(j == 7))
            nc.tensor.matmul(ot[:, 512:1024], wo[:, j, cc, :], bmv[:, 512:1024, j],
                             start=(j == 0), stop=(j == 7))
        nc.vector.tensor_tensor(out=rsd[:, cc, :], in0=ot[:], in1=yprev[:, cc, :],
                                op=ALU.add)
        rsq = wp.tile([128, S_T], bf16, tag="rsq", bufs=2)
        nc.scalar.activation(rsq[:], rsd[:, cc, :], AF.Square)
        pend.append((cc, rsq))
        if len(pend) > 1:
            emit_stats(*pend.pop(0))
    emit_stats(*pend.pop(0))
    scaleb, biasb = _ln_tail(nc, wp, mean_ps, msq_ps, eps_t)
    for cc in range(8):
        nc.vector.tensor_tensor(out=ynext[:, cc, :], in0=rsd[:, cc, :],
                                in1=scaleb[:], op=ALU.mult)
        nc.vector.tensor_tensor(out=ynext[:, cc, :], in0=ynext[:, cc, :],
                                in1=biasb[:], op=ALU.add)
        if gb is not None:
            nc.scalar.activation(ynext[:, cc, :], ynext[:, cc, :], AF.Identity,
                                 scale=gb[gbi][:, cc:cc + 1],
                                 bias=gb[gbi + 1][:, cc:cc + 1])


def _lffn_mm(nc, tc, sb, pf, src, dst, wtile, kn, tn, act):
    """dst[:, t, :] = act(sum_k wtile[:,k,t,:].T @ src[:, k, :]) for t<tn."""
    for t in range(tn):
        ps = pf.tile([128, S_T], f32, tag="facc", bufs=3)
        for k in range(kn):
            nc.tensor.matmul(ps[:, 0:512], wtile[:, k, t, :], src[:, k, 0:512],
                             start=(k == 0), stop=(k == kn - 1))
            nc.tensor.matmul(ps[:, 512:1024], wtile[:, k, t, :], src[:, k, 512:1024],
                             start=(k == 0), stop=(k == kn - 1))
        nc.scalar.activation(dst[:, t, :], ps[:], act)


def _build(affine: bool):
    nc = bacc.Bacc("TRN2", target_bir_lowering=False, debug=False,
                   enable_asserts=True, num_devices=N_CORES)

    def din(name, shape, dt=bf16):
        return nc.dram_tensor(name, list(shape), dt, kind="ExternalInput").ap()

    y0T_d = din("y0T", [BPC, 128, 8, S_T])
    memT_d = din("memT", [BPC, 16, 128, 8 * DQ])
    wq1_d = din("wq1", [128, 8, 8, DQ])
    wkv1_d = din("wkv1", [2, 128, 8, 8, DQ])
    wo1_d = din("wo1", [128, 8, 8, DQ])
    wq2_d = din("wq2", [128, 8, 8, DQ])
    wkv2_d = din("wkv2", [2, 128, 8, 8, DQ])
    wo2_d = din("wo2", [128, 8, 8, DQ])
    e1p_d = din("e1p", [128, 8, 4, DQ])
    d1p_d = din("d1p", [128, 4, 8, DQ])
    e2p_d = din("e2p", [128, 8, 4, DQ])
    d2p_d = din("d2p", [128, 4, 8, DQ])
    mask_d = din("maskneg", [128, DQ], f32)
    ones_d = din("ones", [128, 1])
    if affine:
        grep_d = din("grep", [6, 128, 8], f32)

    outT = nc.dram_tensor("outT", [BPC, 128, 8, S_T], f32,
                          kind="ExternalOutput").ap()

    with tile.TileContext(nc) as tc:
        with tc.tile_pool(name="glob", bufs=1) as gl:
            maskt = gl.tile([128, DQ], f32, tag="maskt")
            nc.sync.dma_start(maskt[:], mask_d[:])
            ones_bf = gl.tile([128, 1], bf16, tag="ones")
            nc.sync.dma_start(ones_bf[:], ones_d[:])
            eps_t = gl.tile([1, 1], f32, tag="eps")
            nc.vector.memset(eps_t[:], EPS)
            gb = None
            if affine:
                gb = [gl.tile([128, 8], f32, tag=f"gb{i}", name=f"gb{i}")
                      for i in range(6)]
                for i in range(6):
                    nc.sync.dma_start(gb[i][:], grep_d[i])

            # rotating activation generations per batch elem (bufs=2)
            def ytile(b, g):
                return gl.tile([128, 8, S_T], bf16, tag=f"yT{b}", bufs=2,
                               name=f"yT{b}_{g}")

            y0 = [ytile(b, 0) for b in range(BPC)]
            for b in range(BPC):
                nc.sync.dma_start(y0[b][:], y0T_d[b])

            # ---- attention phases ----
            def attn(yprev, ynext, memsm, wq_d, wkv_d, wo_d, masked, n_kv, gbi):
                with tc.tile_pool(name="work", bufs=1) as sb:
                    with tc.tile_pool(name="wkv", bufs=1) as wp:
                        wk = wp.tile([128, 8, 8, DQ], bf16, tag="wk")
                        nc.sync.dma_start(wk[:], wkv_d[0])
                        wv = wp.tile([128, 8, 8, DQ], bf16, tag="wv")
                        nc.sync.dma_start(wv[:], wkv_d[1])
                        with tc.tile_pool(name="ps_kv", bufs=1, space="PSUM") as pa:
                            a_sbs = [
                                _kv_stage(nc, tc, sb, wp, pa, yprev[b],
                                          None if memsm is None else memsm[b],
                                          wk, wv, n_kv)
                                for b in range(BPC)]
                    with tc.tile_pool(name="wq", bufs=1) as wp:
                        wq = wp.tile([128, 8, 8, DQ], bf16, tag="wq")
                        nc.sync.dma_start(wq[:], wq_d[:])
                        with tc.tile_pool(name="ps_q", bufs=1, space="PSUM") as pq:
                            bmtps = [
                                _q_stage(nc, tc, sb, wp, pq, yprev[b], a_sbs[b],
                                         wq, masked, maskt, ones_bf)
                                for b in range(BPC)]
                    with tc.tile_pool(name="wo", bufs=1) as wp:
                        wo = wp.tile([128, 8, 8, DQ], bf16, tag="wo")
                        nc.sync.dma_start(wo[:], wo_d[:])
                        with tc.tile_pool(name="ps_o", bufs=1, space="PSUM") as po:
                            for b in range(BPC):
                                _out_stage(nc, tc, sb, wp, po, yprev[b], ynext[b],
                                           bmtps[b], wo, ones_bf, eps_t,
                                           gb=gb, gbi=gbi)

            y1 = [ytile(b, 1) for b in range(BPC)]
            attn(y0, y1, None, wq1_d, wkv1_d, wo1_d, True, 8, 0)
            y2 = [ytile(b, 2) for b in range(BPC)]
            attn(y1, y2, memT_d, wq2_d, wkv2_d, wo2_d, False, 16, 2)

            # ---- LFFN ----
            with tc.tile_pool(name="workf", bufs=1) as sb:
                with tc.tile_pool(name="wf", bufs=1) as wp:
                    e1p = wp.tile([128, 8, 4, DQ], bf16, tag="e1p")
                    nc.sync.dma_start(e1p[:], e1p_d[:])
                    d1p = wp.tile([128, 4, 8, DQ], bf16, tag="d1p")
                    nc.sync.dma_start(d1p[:], d1p_d[:])
                    e2p = wp.tile([128, 8, 4, DQ], bf16, tag="e2p")
                    nc.sync.dma_start(e2p[:], e2p_d[:])
                    d2p = wp.tile([128, 4, 8, DQ], bf16, tag="d2p")
                    nc.sync.dma_start(d2p[:], d2p_d[:])
                    for b in range(BPC):
                        h1T = sb.tile([128, 4, S_T], bf16, tag="h1T", bufs=1)
                        swT = sb.tile([128, 8, S_T], bf16, tag="swT", bufs=1)
                        g1T = sb.tile([128, 4, S_T], bf16, tag="g1T", bufs=1)
                        with tc.tile_pool(name="ps_f", bufs=1, space="PSUM") as pf:
                            _lffn_mm(nc, tc, sb, pf, y2[b], h1T, e1p, 8, 4, AF.Identity)
                            _lffn_mm(nc, tc, sb, pf, h1T, swT, d1p, 4, 8, AF.Silu)
                            _lffn_mm(nc, tc, sb, pf, swT, g1T, e2p, 8, 4, AF.Identity)
                        with tc.tile_pool(name="ps_f2", bufs=1, space="PSUM") as po:
                            # D2 matmul + residual + LN3 (writes outf f32)
                            rsd = sb.tile([128, 8, S_T], bf16, tag="rsd", bufs=1)
                            mean_ps = po.tile([1, S_T], f32, tag="mean", bufs=1)
                            msq_ps = po.tile([1, S_T], f32, tag="msq", bufs=1)
                            def emit_stats(cc, rsq):
                                nc.tensor.matmul(mean_ps[:, 0:512], ones_bf[:],
                                                 rsd[:, cc, 0:512],
                                                 start=(cc == 0), stop=(cc == 7))
                                nc.tensor.matmul(mean_ps[:, 512:1024], ones_bf[:],
                                                 rsd[:, cc, 512:1024],
                                                 start=(cc == 0), stop=(cc == 7))
                                nc.tensor.matmul(msq_ps[:, 0:512], ones_bf[:],
                                                 rsq[:, 0:512],
                                                 start=(cc == 0), stop=(cc == 7))
                                nc.tensor.matmul(msq_ps[:, 512:1024], ones_bf[:],
                                                 rsq[:, 512:1024],
                                                 start=(cc == 0), stop=(cc == 7))
                            pend = []
                            for cc in range(8):
                                ps = po.tile([128, S_T], f32, tag="ot", bufs=2)
                                for k in range(4):
                                    nc.tensor.matmul(
                                        ps[:, 0:512], d2p[:, k, cc, :],
                                        g1T[:, k, 0:512],
                                        start=(k == 0), stop=(k == 3))
                                    nc.tensor.matmul(
                                        ps[:, 512:1024], d2p[:, k, cc, :],
                                        g1T[:, k, 512:1024],
                                        start=(k == 0), stop=(k == 3))
                                nc.vector.tensor_tensor(
                                    out=rsd[:, cc, :], in0=ps[:],
                                    in1=y2[b][:, cc, :], op=ALU.add)
                                rsq = sb.tile([128, S_T], bf16, tag="rsq", bufs=2)
                                nc.scalar.activation(rsq[:], rsd[:, cc, :], AF.Square)
                                pend.append((cc, rsq))
                                if len(pend) > 1:
                                    emit_stats(*pend.pop(0))
                            emit_stats(*pend.pop(0))
                            scaleb, biasb = _ln_tail(nc, sb, mean_ps, msq_ps, eps_t)
                            for cc in range(8):
                                outf = sb.tile([128, S_T], f32, tag="outf", bufs=2)
                                nc.vector.tensor_tensor(
                                    out=outf[:], in0=rsd[:, cc, :],
                                    in1=scaleb[:], op=ALU.mult)
                                nc.vector.tensor_tensor(
                                    out=outf[:], in0=outf[:],
                                    in1=biasb[:], op=ALU.add)
                                if gb is not None:
                                    nc.scalar.activation(
                                        outf[:], outf[:], AF.Identity,
                                        scale=gb[4][:, cc:cc + 1],
                                        bias=gb[5][:, cc:cc + 1])
                                nc.sync.dma_start(outT[b][:, cc, :], outf[:])

    nc.compile()
    return nc


_CACHE = {}


def _prep_host(inputs):
    g = {k: np.asarray(v) for k, v in inputs.items()}
    affine = not (
        np.all(g["g1"] == 1) and np.all(g["g2"] == 1) and np.all(g["g3"] == 1)
        and np.all(g["b1"] == 0) and np.all(g["b2"] == 0) and np.all(g["b3"] == 0))

    def wq_pack(w):  # stationary: [128, h, k, dq]
        return np.ascontiguousarray(
            w.transpose(1, 0, 2).reshape(8, 128, 8, DQ).transpose(1, 2, 0, 3)
        ).astype(bf)

    def wkv_pack(wk_, wv_):  # moving: [2, 128, k, h, dq]
        def one(w):
            # w [H, D, DQ]: arr[p, k, h, :] = w[h, 128k+p, :]
            return w.transpose(1, 0, 2).reshape(8, 128, 8, DQ).transpose(1, 0, 2, 3)
        return np.ascontiguousarray(np.stack([one(wk_), one(wv_)])).astype(bf)

    def lhsT_pack(wT, kn, tn):  # [128, k, t, 128] from wT [kn*128, tn*128]
        return np.ascontiguousarray(
            wT.reshape(kn, 128, tn, DQ).transpose(1, 0, 2, 3)).astype(bf)

    host = {}
    host["wq1"] = wq_pack(g["Wq1"])
    host["wkv1"] = wkv_pack(g["Wk1"], g["Wv1"])
    host["wo1"] = lhsT_pack(np.ascontiguousarray(g["Wo1"].T), 8, 8)
    host["wq2"] = wq_pack(g["Wq2"])
    host["wkv2"] = wkv_pack(g["Wk2"], g["Wv2"])
    host["wo2"] = lhsT_pack(np.ascontiguousarray(g["Wo2"].T), 8, 8)
    host["e1p"] = lhsT_pack(np.ascontiguousarray(g["E1"].T), 8, 4)
    host["d1p"] = lhsT_pack(np.ascontiguousarray(g["D1"].T), 4, 8)
    host["e2p"] = lhsT_pack(np.ascontiguousarray(g["E2"].T), 8, 4)
    host["d2p"] = lhsT_pack(np.ascontiguousarray(g["D2"].T), 4, 8)
    host["maskneg"] = np.where(
        np.arange(128)[:, None] <= np.arange(DQ)[None, :], 0.0, NEG
    ).astype(np.float32)
    host["ones"] = np.ones((128, 1), np.float32).astype(bf)
    if affine:
        host["grep"] = np.stack([
            g[n].astype(np.float32).reshape(8, 128).T
            for n in ("g1", "b1", "g2", "b2", "g3", "b3")]).copy()

    in_maps = []
    y = g["y"].astype(np.float32)
    mem = g["mem"].astype(np.float32)
    for c in range(N_CORES):
        sl = slice(BPC * c, BPC * (c + 1))
        m = dict(host)
        # y0T [b][p, k, s]: = y[b, s, 128k+p]
        yT_ = y[sl].transpose(0, 2, 1)  # [b, D, S]
        m["y0T"] = np.ascontiguousarray(
            yT_.reshape(BPC, 8, 128, S_T).transpose(0, 2, 1, 3)).astype(bf)
        # memT [b][sm][p, 128k+sigma] = mem[b, 128sm+sigma, 128k+p]
        mm = mem[sl].reshape(BPC, 16, 128, 8, 128)  # [b, sm, sigma, k, p]
        m["memT"] = np.ascontiguousarray(
            mm.transpose(0, 1, 4, 3, 2).reshape(BPC, 16, 128, 8 * DQ)).astype(bf)
        in_maps.append(m)
    return in_maps, affine


def kernel(**inputs):
    in_maps, affine = _prep_host(inputs)
    if affine not in _CACHE:
        _CACHE[affine] = _build(affine)
    nc = _CACHE[affine]
    res = run_bass_kernel_spmd(nc, in_maps, list(range(N_CORES)))
    # outT [BPC, 128, 8, S_T] -> out[b, s, 128cc+p] = outT[b, p, cc, s]
    outs = []
    for r in res.results:
        o = r["outT"].reshape(BPC, 128, 8, S_T).transpose(0, 3, 2, 1)
        outs.append(np.ascontiguousarray(o.reshape(BPC, S_T, D)))
    return np.concatenate(outs, axis=0)


# revision 4
# speedup vs baseline: 1.0600x; 1.0600x over previous
# Trainium2 Bass kernel for nn_DecoderBlock — transposed-activation design.
#
# Sharding: data-parallel over batch — 16 elems / 8 cores = 2 per core.
#
# All activations live in SBUF in TRANSPOSED layout xT [D(part, 8 chunks), S]
# for the whole kernel; natural layout is never materialized on device (the
# host transposes the final output back, which is free).
#
# Per head h:
#   QT = Wq[h].T @ xT            (lhsT = Wq chunks, rhs = xT)      [dq, S]
#   expQT = exp((QT + mask)/sc)  -> qs = colsum via ones-matmul    [1, S]
#   softQT = expQT * bcast(1/qs) (GPSIMD partition_broadcast)
#   K/V natural per sm-tile: lhsT = xT s-chunk, rhs = Wk/Wv packed [s, 8*dq]
#   expK, V' = V / rowsum(expK); A[h] += expK[h].T @ V'[h]         [dq, dq]
#   BmT[h] = A[h].T @ softQT                                       [dq, S]
# Output (torch .view(b,w,h*d) quirk folded into a strided AP):
#   outT[cc] = sum_j WoT[j,cc].T @ BmTpack[:, w', j]   (w' = 128h+q, s = 8q+j)
# Residual + LayerNorm in transposed layout: stats over the partition axis
# via ones-matmuls, per-column scale/bias replicated with partition_broadcast.
# LFFN fully transposed: h1T = E1 @ yT, h2T = D1 @ h1T, silu, E2, D2.
import numpy as np
import ml_dtypes

import concourse.bacc as bacc
import concourse.mybir as mybir
import concourse.tile as tile
from concourse.bass_utils import run_bass_kernel_spmd

H, D, DQ, BNK, HID = 8, 1024, 128, 512, 1024
B, S_T, S_M = 16, 1024, 2048
SCALE = DQ ** 0.25
EPS = 1e-5
NEG = -200.0
N_CORES = 8
BPC = B // N_CORES

f32 = mybir.dt.float32
bf16 = mybir.dt.bfloat16
AF = mybir.ActivationFunctionType
ALU = mybir.AluOpType
bf = ml_dtypes.bfloat16


def _ln_tail(nc, sb, mean_ps, msq_ps, eps_t):
    """Column stats [1,S] -> broadcast scale/bias tiles [128,S] f32."""
    mu = sb.tile([1, S_T], f32, tag="ln_t", bufs=4, name="mu")
    nc.scalar.activation(mu[:], mean_ps[:], AF.Identity, scale=1.0 / D)
    ex2 = sb.tile([1, S_T], f32, tag="ln_t", bufs=4, name="ex2")
    nc.scalar.activation(ex2[:], msq_ps[:], AF.Identity, scale=1.0 / D)
    var = sb.tile([1, S_T], f32, tag="ln_t", bufs=4, name="var")
    nc.vector.tensor_tensor(out=var[:], in0=mu[:], in1=mu[:], op=ALU.mult)
    nc.vector.tensor_tensor(out=var[:], in0=ex2[:], in1=var[:], op=ALU.subtract)
    sd = sb.tile([1, S_T], f32, tag="ln_t", bufs=4, name="sd")
    nc.scalar.activation(sd[:], var[:], AF.Sqrt, bias=eps_t[:])
    rstd = sb.tile([1, S_T], f32, tag="ln_r", bufs=2, name="rstd")
    nc.vector.reciprocal(rstd[:], sd[:])
    nmu = sb.tile([1, S_T], f32, tag="ln_r", bufs=2, name="nmu")
    nc.vector.scalar_tensor_tensor(
        out=nmu[:], in0=mu[:], scalar=-1.0, in1=rstd[:],
        op0=ALU.mult, op1=ALU.mult)
    scaleb = sb.tile([128, S_T], f32, tag="ln_scaleb", bufs=1)
    nc.gpsimd.partition_broadcast(scaleb[:], rstd[:])
    biasb = sb.tile([128, S_T], f32, tag="ln_biasb", bufs=1)
    nc.gpsimd.partition_broadcast(biasb[:], nmu[:])
    return scaleb, biasb


def _kv_stage(nc, tc, sb, wp, pa, yprev, memsm_dram, wk, wv, n_kv):
    """K/V natural projections + A accumulation for one batch elem.
    Returns a_sb [128, 8, DQ] bf16 (A per head). A matmuls run one sm-tile
    behind the projections so the PE never waits on the evac chain."""
    a_sb = sb.tile([128, 8, DQ], bf16, tag="a_sb", bufs=2)
    a_acc = sb.tile([128, 1024], f32, tag="a_acc", bufs=2)
    pend = []

    # NOTE: psum accumulation groups must not interleave within one bank, so
    # each sm's A-partial is a single-shot matmul set, accumulated on the DVE.
    def emit_a(sm, ek, ev):
        apart = pa.tile([128, 1024], f32, tag="apart", bufs=1)
        for h in range(H):
            nc.tensor.matmul(apart[:, DQ * h:DQ * (h + 1)],
                             ek[:, h, :], ev[:, h, :])
        if sm == 0:
            nc.vector.tensor_copy(a_acc[:], apart[:])
        else:
            nc.vector.tensor_tensor(out=a_acc[:], in0=a_acc[:], in1=apart[:],
                                    op=ALU.add)

    for sm in range(n_kv):
        if memsm_dram is None:
            def lhsT(k, sm=sm):
                return yprev[:, k, DQ * sm:DQ * (sm + 1)]
        else:
            mt = wp.tile([128, 8, DQ], bf16, tag="memsm", bufs=4)
            nc.sync.dma_start(mt[:], memsm_dram[sm])
            def lhsT(k, mt=mt):
                return mt[:, k, :]
        klo = pa.tile([128, 512], f32, tag="kv", bufs=6, name="klo")
        khi = pa.tile([128, 512], f32, tag="kv", bufs=6, name="khi")
        vlo = pa.tile([128, 512], f32, tag="kv", bufs=6, name="vlo")
        vhi = pa.tile([128, 512], f32, tag="kv", bufs=6, name="vhi")
        for k in range(8):
            lt = lhsT(k)
            nc.tensor.matmul(klo[:], lt, wk[:, k, 0:4, :], start=(k == 0), stop=(k == 7))
            nc.tensor.matmul(khi[:], lt, wk[:, k, 4:8, :], start=(k == 0), stop=(k == 7))
            nc.tensor.matmul(vlo[:], lt, wv[:, k, 0:4, :], start=(k == 0), stop=(k == 7))
            nc.tensor.matmul(vhi[:], lt, wv[:, k, 4:8, :], start=(k == 0), stop=(k == 7))
        expk = wp.tile([128, 8, DQ], bf16, tag="expk", bufs=3)
        nc.scalar.activation(expk[:, 0:4, :], klo[:], AF.Exp, scale=1.0 / SCALE)
        nc.scalar.activation(expk[:, 4:8, :], khi[:], AF.Exp, scale=1.0 / SCALE)
        krs = wp.tile([128, 8], f32, tag="krs", bufs=2)
        nc.vector.tensor_reduce(out=krs[:], in_=expk[:],
                                axis=mybir.AxisListType.X, op=ALU.add)
        krr = wp.tile([128, 8], f32, tag="krr", bufs=2)
        nc.vector.reciprocal(krr[:], krs[:])
        expv = wp.tile([128, 8, DQ], bf16, tag="expv", bufs=3)
        nc.vector.tensor_tensor(
            out=expv[:, 0:4, :], in0=vlo[:].rearrange("p (h q) -> p h q", h=4),
            in1=krr[:, 0:4].unsqueeze(2).broadcast_to([128, 4, DQ]), op=ALU.mult)
        nc.vector.tensor_tensor(
            out=expv[:, 4:8, :], in0=vhi[:].rearrange("p (h q) -> p h q", h=4),
            in1=krr[:, 4:8].unsqueeze(2).broadcast_to([128, 4, DQ]), op=ALU.mult)
        pend.append((sm, expk, expv))
        if len(pend) > 1:
            emit_a(*pend.pop(0))
    emit_a(*pend.pop(0))
    nc.vector.tensor_copy(a_sb[:].rearrange("p h q -> p (h q)"), a_acc[:])
    return a_sb


def _q_stage(nc, tc, sb, wp, pq, yprev, a_sb, wq, masked, maskt, ones_bf):
    """Q proj + softmax + BmT for one batch elem -> bmtp [128, 8, S_T].
    Pass-structured so the PE stream never waits on the softmax chain."""
    bmtp = sb.tile([128, 8, S_T], bf16, tag="bmtp", bufs=2)
    expqs = []
    for h in range(H):
        qt = pq.tile([128, S_T], f32, tag="qt", bufs=3)
        for k in range(8):
            nc.tensor.matmul(qt[:, 0:512], wq[:, h, k, :], yprev[:, k, 0:512],
                             start=(k == 0), stop=(k == 7))
            nc.tensor.matmul(qt[:, 512:1024], wq[:, h, k, :], yprev[:, k, 512:1024],
                             start=(k == 0), stop=(k == 7))
        if masked:
            nc.vector.tensor_tensor(out=qt[:, 0:DQ], in0=qt[:, 0:DQ],
                                    in1=maskt[:], op=ALU.add)
        expq = wp.tile([128, S_T], bf16, tag="expq", bufs=8, name=f"expq{h}")
        nc.scalar.activation(expq[:], qt[:], AF.Exp, scale=1.0 / SCALE)
        expqs.append(expq)
    for h in range(H):
        qs = pq.tile([1, S_T], f32, tag="qs", bufs=1)
        nc.tensor.matmul(qs[:, 0:512], ones_bf[:], expqs[h][:, 0:512])
        nc.tensor.matmul(qs[:, 512:1024], ones_bf[:], expqs[h][:, 512:1024])
        qsr = wp.tile([1, S_T], f32, tag="qsr", bufs=2)
        nc.vector.reciprocal(qsr[:], qs[:])
        qsb = wp.tile([128, S_T], f32, tag="qsb", bufs=2)
        nc.gpsimd.partition_broadcast(qsb[:], qsr[:])
        nc.vector.tensor_tensor(out=expqs[h][:], in0=expqs[h][:], in1=qsb[:],
                                op=ALU.mult)
    for h in range(H):
        bmt = pq.tile([128, S_T], f32, tag="qt", bufs=3)
        nc.tensor.matmul(bmt[:, 0:512], a_sb[:, h, :], expqs[h][:, 0:512])
        nc.tensor.matmul(bmt[:, 512:1024], a_sb[:, h, :], expqs[h][:, 512:1024])
        nc.scalar.activation(bmtp[:, h, :], bmt[:], AF.Identity)
    return bmtp


def _out_stage(nc, tc, sb, wp, po, yprev, ynext, bmtp, wo, ones_bf, eps_t,
               out_f32=False, gb=None, gbi=0):
    """Wo matmul (transposed out) + residual + LN for one batch elem.
    Stats matmuls run one cc-tile behind so the PE never waits."""
    bmv = bmtp[:].rearrange("p h (m e) -> p (h m) e", e=8)
    rsd = wp.tile([128, 8, S_T], bf16, tag="rsd", bufs=1)
    mean_ps = po.tile([1, S_T], f32, tag="mean", bufs=1)
    msq_ps = po.tile([1, S_T], f32, tag="msq", bufs=1)
    pend = []

    def emit_stats(cc, rsq):
        nc.tensor.matmul(mean_ps[:, 0:512], ones_bf[:], rsd[:, cc, 0:512],
                         start=(cc == 0), stop=(cc == 7))
        nc.tensor.matmul(mean_ps[:, 512:1024], ones_bf[:], rsd[:, cc, 512:1024],
                         start=(cc == 0), stop=(cc == 7))
        nc.tensor.matmul(msq_ps[:, 0:512], ones_bf[:], rsq[:, 0:512],
                         start=(cc == 0), stop=(cc == 7))
        nc.tensor.matmul(msq_ps[:, 512:1024], ones_bf[:], rsq[:, 512:1024],
                         start=(cc == 0), stop=(cc == 7))

    for cc in range(8):
        ot = po.tile([128, S_T], f32, tag="ot", bufs=2)
        for j in range(8):
            nc.tensor.matmul(ot[:, 0:512], wo[:, j, cc, :], bmv[:, 0:512, j],
                             start=(j == 0), stop=(j == 7))
            nc.tensor.matmul(ot[:, 512:1024], wo[:, j, cc, :], bmv[:, 512:1024, j],
                             start=(j == 0), stop=(j == 7))
        nc.vector.tensor_tensor(out=rsd[:, cc, :], in0=ot[:], in1=yprev[:, cc, :],
                                op=ALU.add)
        rsq = wp.tile([128, S_T], bf16, tag="rsq", bufs=2)
        nc.scalar.activation(rsq[:], rsd[:, cc, :], AF.Square)
        pend.append((cc, rsq))
        if len(pend) > 1:
            emit_stats(*pend.pop(0))
    emit_stats(*pend.pop(0))
    scaleb, biasb = _ln_tail(nc, wp, mean_ps, msq_ps, eps_t)
    for cc in range(8):
        nc.vector.tensor_tensor(out=ynext[:, cc, :], in0=rsd[:, cc, :],
                                in1=scaleb[:], op=ALU.mult)
        nc.vector.tensor_tensor(out=ynext[:, cc, :], in0=ynext[:, cc, :],
                                in1=biasb[:], op=ALU.add)
        if gb is not None:
            nc.scalar.activation(ynext[:, cc, :], ynext[:, cc, :], AF.Identity,
                                 scale=gb[gbi][:, cc:cc + 1],
                                 bias=gb[gbi + 1][:, cc:cc + 1])


def _lffn_mm(nc, tc, sb, pf, src, dst, wtile, kn, tn, act):
    """dst[:, t, :] = act(sum_k wtile[:,k,t,:].T @ src[:, k, :]) for t<tn."""
    for t in range(tn):
        ps = pf.tile([128, S_T], f32, tag="facc", bufs=3)
        for k in range(kn):
            nc.tensor.matmul(ps[:, 0:512], wtile[:, k, t, :], src[:, k, 0:512],
                             start=(k == 0), stop=(k == kn - 1))
            nc.tensor.matmul(ps[:, 512:1024], wtile[:, k, t, :], src[:, k, 512:1024],
                             start=(k == 0), stop=(k == kn - 1))
        nc.scalar.activation(dst[:, t, :], ps[:], act)


def _build(affine: bool):
    nc = bacc.Bacc("TRN2", target_bir_lowering=False, debug=False,
                   enable_asserts=True, num_devices=N_CORES)

    def din(name, shape, dt=bf16):
        return nc.dram_tensor(name, list(shape), dt, kind="ExternalInput").ap()

    y0T_d = din("y0T", [BPC, 128, 8, S_T])
    memT_d = din("memT", [BPC, 16, 128, 8 * DQ])
    wq1_d = din("wq1", [128, 8, 8, DQ])
    wkv1_d = din("wkv1", [2, 128, 8, 8, DQ])
    wo1_d = din("wo1", [128, 8, 8, DQ])
    wq2_d = din("wq2", [128, 8, 8, DQ])
    wkv2_d = din("wkv2", [2, 128, 8, 8, DQ])
    wo2_d = din("wo2", [128, 8, 8, DQ])
    e1p_d = din("e1p", [128, 8, 4, DQ])
    d1p_d = din("d1p", [128, 4, 8, DQ])
    e2p_d = din("e2p", [128, 8, 4, DQ])
    d2p_d = din("d2p", [128, 4, 8, DQ])
    mask_d = din("maskneg", [128, DQ], f32)
    ones_d = din("ones", [128, 1])
    if affine:
        grep_d = din("grep", [6, 128, 8], f32)

    outT = nc.dram_tensor("outT", [BPC, 128, 8, S_T], f32,
                          kind="ExternalOutput").ap()

    with tile.TileContext(nc) as tc:
        with tc.tile_pool(name="glob", bufs=1) as gl:
            maskt = gl.tile([128, DQ], f32, tag="maskt")
            nc.sync.dma_start(maskt[:], mask_d[:])
            ones_bf = gl.tile([128, 1], bf16, tag="ones")
            nc.sync.dma_start(ones_bf[:], ones_d[:])
            eps_t = gl.tile([1, 1], f32, tag="eps")
            nc.vector.memset(eps_t[:], EPS)
            gb = None
            if affine:
                gb = [gl.tile([128, 8], f32, tag=f"gb{i}", name=f"gb{i}")
                      for i in range(6)]
                for i in range(6):
                    nc.sync.dma_start(gb[i][:], grep_d[i])

            # rotating activation generations per batch elem (bufs=2)
            def ytile(b, g):
                return gl.tile([128, 8, S_T], bf16, tag=f"yT{b}", bufs=2,
                               name=f"yT{b}_{g}")

            y0 = [ytile(b, 0) for b in range(BPC)]
            nc.sync.dma_start(y0[0][:], y0T_d[0])

            # ---- attention phases ----
            # wq loads at phase start (overlaps KV stage); wo loads at Q-stage
            # start (overlaps Q); wk/wv freed before the Q stage runs.
            def attn(yprev, ynext, memsm, wq_d, wkv_d, wo_d, masked, n_kv, gbi,
                     prefetch=()):
                with tc.tile_pool(name="work", bufs=1) as sb, \
                     tc.tile_pool(name="wqp", bufs=1) as wqp:
                    wq = wqp.tile([128, 8, 8, DQ], bf16, tag="wq")
                    nc.sync.dma_start(wq[:], wq_d[:])
                    for dst, srcd in prefetch:
                        nc.sync.dma_start(dst[:], srcd)
                    with tc.tile_pool(name="wkvp", bufs=1) as wkvp:
                        wk = wkvp.tile([128, 8, 8, DQ], bf16, tag="wk")
                        nc.sync.dma_start(wk[:], wkv_d[0])
                        wv = wkvp.tile([128, 8, 8, DQ], bf16, tag="wv")
                        nc.sync.dma_start(wv[:], wkv_d[1])
                        with tc.tile_pool(name="kvt", bufs=1) as kvt:
                            with tc.tile_pool(name="ps_kv", bufs=1, space="PSUM") as pa:
                                a_sbs = [
                                    _kv_stage(nc, tc, sb, kvt, pa, yprev[b],
                                              None if memsm is None else memsm[b],
                                              wk, wv, n_kv)
                                    for b in range(BPC)]
                    with tc.tile_pool(name="wop", bufs=1) as wop:
                        wo = wop.tile([128, 8, 8, DQ], bf16, tag="wo")
                        nc.sync.dma_start(wo[:], wo_d[:])
                        with tc.tile_pool(name="qtp", bufs=1) as qtp:
                            with tc.tile_pool(name="ps_q", bufs=1, space="PSUM") as pq:
                                bmtps = [
                                    _q_stage(nc, tc, sb, qtp, pq, yprev[b], a_sbs[b],
                                             wq, masked, maskt, ones_bf)
                                    for b in range(BPC)]
                        with tc.tile_pool(name="otp", bufs=1) as otp:
                            with tc.tile_pool(name="ps_o", bufs=1, space="PSUM") as po:
                                for b in range(BPC):
                                    _out_stage(nc, tc, sb, otp, po, yprev[b], ynext[b],
                                               bmtps[b], wo, ones_bf, eps_t,
                                               gb=gb, gbi=gbi)

            y1 = [ytile(b, 1) for b in range(BPC)]
            attn(y0, y1, None, wq1_d, wkv1_d, wo1_d, True, 8, 0,
                 prefetch=[(y0[b], y0T_d[b]) for b in range(1, BPC)])
            y2 = [ytile(b, 2) for b in range(BPC)]
            attn(y1, y2, memT_d, wq2_d, wkv2_d, wo2_d, False, 16, 2)

            # ---- LFFN ----
            with tc.tile_pool(name="workf", bufs=1) as sb:
                with tc.tile_pool(name="wf", bufs=1) as wp:
                    e1p = wp.tile([128, 8, 4, DQ], bf16, tag="e1p")
                    nc.sync.dma_start(e1p[:], e1p_d[:])
                    d1p = wp.tile([128, 4, 8, DQ], bf16, tag="d1p")
                    nc.sync.dma_start(d1p[:], d1p_d[:])
                    e2p = wp.tile([128, 8, 4, DQ], bf16, tag="e2p")
                    nc.sync.dma_start(e2p[:], e2p_d[:])
                    d2p = wp.tile([128, 4, 8, DQ], bf16, tag="d2p")
                    nc.sync.dma_start(d2p[:], d2p_d[:])
                    for b in range(BPC):
                        h1T = sb.tile([128, 4, S_T], bf16, tag="h1T", bufs=1)
                        swT = sb.tile([128, 8, S_T], bf16, tag="swT", bufs=1)
                        g1T = sb.tile([128, 4, S_T], bf16, tag="g1T", bufs=1)
                        with tc.tile_pool(name="ps_f", bufs=1, space="PSUM") as pf:
                            _lffn_mm(nc, tc, sb, pf, y2[b], h1T, e1p, 8, 4, AF.Identity)
                            _lffn_mm(nc, tc, sb, pf, h1T, swT, d1p, 4, 8, AF.Silu)
                            _lffn_mm(nc, tc, sb, pf, swT, g1T, e2p, 8, 4, AF.Identity)
                        with tc.tile_pool(name="ps_f2", bufs=1, space="PSUM") as po:
                            # D2 matmul + residual + LN3 (writes outf f32)
                            rsd = sb.tile([128, 8, S_T], bf16, tag="rsd", bufs=1)
                            mean_ps = po.tile([1, S_T], f32, tag="mean", bufs=1)
                            msq_ps = po.tile([1, S_T], f32, tag="msq", bufs=1)
                            def emit_stats(cc, rsq):
                                nc.tensor.matmul(mean_ps[:, 0:512], ones_bf[:],
                                                 rsd[:, cc, 0:512],
                                                 start=(cc == 0), stop=(cc == 7))
                                nc.tensor.matmul(mean_ps[:, 512:1024], ones_bf[:],
                                                 rsd[:, cc, 512:1024],
                                                 start=(cc == 0), stop=(cc == 7))
                                nc.tensor.matmul(msq_ps[:, 0:512], ones_bf[:],
                                                 rsq[:, 0:512],
                                                 start=(cc == 0), stop=(cc == 7))
                                nc.tensor.matmul(msq_ps[:, 512:1024], ones_bf[:],
                                                 rsq[:, 512:1024],
                                                 start=(cc == 0), stop=(cc == 7))
                            pend = []
                            for cc in range(8):
                                ps = po.tile([128, S_T], f32, tag="ot", bufs=2)
                                for k in range(4):
                                    nc.tensor.matmul(
                                        ps[:, 0:512], d2p[:, k, cc, :],
                                        g1T[:, k, 0:512],
                                        start=(k == 0), stop=(k == 3))
                                    nc.tensor.matmul(
                                        ps[:, 512:1024], d2p[:, k, cc, :],
                                        g1T[:, k, 512:1024],
                                        start=(k == 0), stop=(k == 3))
                                nc.vector.tensor_tensor(
                                    out=rsd[:, cc, :], in0=ps[:],
                                    in1=y2[b][:, cc, :], op=ALU.add)
                                rsq = sb.tile([128, S_T], bf16, tag="rsq", bufs=2)
                                nc.scalar.activation(rsq[:], rsd[:, cc, :], AF.Square)
                                pend.append((cc, rsq))
                                if len(pend) > 1:
                                    emit_stats(*pend.pop(0))
                            emit_stats(*pend.pop(0))
                            scaleb, biasb = _ln_tail(nc, sb, mean_ps, msq_ps, eps_t)
                            for cc in range(8):
                                outf = sb.tile([128, S_T], f32, tag="outf", bufs=2)
                                nc.vector.tensor_tensor(
                                    out=outf[:], in0=rsd[:, cc, :],
                                    in1=scaleb[:], op=ALU.mult)
                                nc.vector.tensor_tensor(
                                    out=outf[:], in0=outf[:],
                                    in1=biasb[:], op=ALU.add)
                                if gb is not None:
                                    nc.scalar.activation(
                                        outf[:], outf[:], AF.Identity,
                                        scale=gb[4][:, cc:cc + 1],
                                        bias=gb[5][:, cc:cc + 1])
                                nc.sync.dma_start(outT[b][:, cc, :], outf[:])

    nc.compile()
    return nc


_CACHE = {}


def _prep_host(inputs):
    g = {k: np.asarray(v) for k, v in inputs.items()}
    affine = not (
        np.all(g["g1"] == 1) and np.all(g["g2"] == 1) and np.all(g["g3"] == 1)
        and np.all(g["b1"] == 0) and np.all(g["b2"] == 0) and np.all(g["b3"] == 0))

    def wq_pack(w):  # stationary: [128, h, k, dq]
        return np.ascontiguousarray(
            w.transpose(1, 0, 2).reshape(8, 128, 8, DQ).transpose(1, 2, 0, 3)
        ).astype(bf)

    def wkv_pack(wk_, wv_):  # moving: [2, 128, k, h, dq]
        def one(w):
            # w [H, D, DQ]: arr[p, k, h, :] = w[h, 128k+p, :]
            return w.transpose(1, 0, 2).reshape(8, 128, 8, DQ).transpose(1, 0, 2, 3)
        return np.ascontiguousarray(np.stack([one(wk_), one(wv_)])).astype(bf)

    def lhsT_pack(wT, kn, tn):  # [128, k, t, 128] from wT [kn*128, tn*128]
        return np.ascontiguousarray(
            wT.reshape(kn, 128, tn, DQ).transpose(1, 0, 2, 3)).astype(bf)

    host = {}
    host["wq1"] = wq_pack(g["Wq1"])
    host["wkv1"] = wkv_pack(g["Wk1"], g["Wv1"])
    host["wo1"] = lhsT_pack(np.ascontiguousarray(g["Wo1"].T), 8, 8)
    host["wq2"] = wq_pack(g["Wq2"])
    host["wkv2"] = wkv_pack(g["Wk2"], g["Wv2"])
    host["wo2"] = lhsT_pack(np.ascontiguousarray(g["Wo2"].T), 8, 8)
    host["e1p"] = lhsT_pack(np.ascontiguousarray(g["E1"].T), 8, 4)
    host["d1p"] = lhsT_pack(np.ascontiguousarray(g["D1"].T), 4, 8)
    host["e2p"] = lhsT_pack(np.ascontiguousarray(g["E2"].T), 8, 4)
    host["d2p"] = lhsT_pack(np.ascontiguousarray(g["D2"].T), 4, 8)
    host["maskneg"] = np.where(
        np.arange(128)[:, None] <= np.arange(DQ)[None, :], 0.0, NEG
    ).astype(np.float32)
    host["ones"] = np.ones((128, 1), np.float32).astype(bf)
    if affine:
        host["grep"] = np.stack([
            g[n].astype(np.float32).reshape(8, 128).T
            for n in ("g1", "b1", "g2", "b2", "g3", "b3")]).copy()

    in_maps = []
    y = g["y"].astype(np.float32)
    mem = g["mem"].astype(np.float32)
    for c in range(N_CORES):
        sl = slice(BPC * c, BPC * (c + 1))
        m = dict(host)
        # y0T [b][p, k, s]: = y[b, s, 128k+p]
        yT_ = y[sl].transpose(0, 2, 1)  # [b, D, S]
        m["y0T"] = np.ascontiguousarray(
            yT_.reshape(BPC, 8, 128, S_T).transpose(0, 2, 1, 3)).astype(bf)
        # memT [b][sm][p, 128k+sigma] = mem[b, 128sm+sigma, 128k+p]
        mm = mem[sl].reshape(BPC, 16, 128, 8, 128)  # [b, sm, sigma, k, p]
        m["memT"] = np.ascontiguousarray(
            mm.transpose(0, 1, 4, 3, 2).reshape(BPC, 16, 128, 8 * DQ)).astype(bf)
        in_maps.append(m)
    return in_maps, affine


def kernel(**inputs):
    in_maps, affine = _prep_host(inputs)
    if affine not in _CACHE:
        _CACHE[affine] = _build(affine)
    nc = _CACHE[affine]
    res = run_bass_kernel_spmd(nc, in_maps, list(range(N_CORES)))
    # outT [BPC, 128, 8, S_T] -> out[b, s, 128cc+p] = outT[b, p, cc, s]
    outs = []
    for r in res.results:
        o = r["outT"].reshape(BPC, 128, 8, S_T).transpose(0, 3, 2, 1)
        outs.append(np.ascontiguousarray(o.reshape(BPC, S_T, D)))
    return np.concatenate(outs, axis=0)


# revision 5
# speedup vs baseline: 1.0944x; 1.0324x over previous
# Trainium2 Bass kernel for nn_DecoderBlock — transposed-activation design.
#
# Sharding: data-parallel over batch — 16 elems / 8 cores = 2 per core.
#
# All activations live in SBUF in TRANSPOSED layout xT [D(part, 8 chunks), S]
# for the whole kernel; natural layout is never materialized on device (the
# host transposes the final output back, which is free).
#
# Per head h:
#   QT = Wq[h].T @ xT            (lhsT = Wq chunks, rhs = xT)      [dq, S]
#   expQT = exp((QT + mask)/sc)  -> qs = colsum via ones-matmul    [1, S]
#   softQT = expQT * bcast(1/qs) (GPSIMD partition_broadcast)
#   K/V natural per sm-tile: lhsT = xT s-chunk, rhs = Wk/Wv packed [s, 8*dq]
#   expK, V' = V / rowsum(expK); A[h] += expK[h].T @ V'[h]         [dq, dq]
#   BmT[h] = A[h].T @ softQT                                       [dq, S]
# Output (torch .view(b,w,h*d) quirk folded into a strided AP):
#   outT[cc] = sum_j WoT[j,cc].T @ BmTpack[:, w', j]   (w' = 128h+q, s = 8q+j)
# Residual + LayerNorm in transposed layout: stats over the partition axis
# via ones-matmuls, per-column scale/bias replicated with partition_broadcast.
# LFFN fully transposed: h1T = E1 @ yT, h2T = D1 @ h1T, silu, E2, D2.
import numpy as np
import ml_dtypes

import concourse.bacc as bacc
import concourse.mybir as mybir
import concourse.tile as tile
from concourse.bass_utils import run_bass_kernel_spmd

H, D, DQ, BNK, HID = 8, 1024, 128, 512, 1024
B, S_T, S_M = 16, 1024, 2048
SCALE = DQ ** 0.25
EPS = 1e-5
NEG = -200.0
N_CORES = 8
BPC = B // N_CORES

f32 = mybir.dt.float32
bf16 = mybir.dt.bfloat16
AF = mybir.ActivationFunctionType
ALU = mybir.AluOpType
bf = ml_dtypes.bfloat16


def _ln_tail(nc, sb, mean_ps, msq_ps, eps_t, scaleb, biasb):
    """Column stats [1,S] -> broadcast scale/bias into the given tiles."""
    mu = sb.tile([1, S_T], f32, tag="ln_t", bufs=4, name="mu")
    nc.scalar.activation(mu[:], mean_ps[:], AF.Identity, scale=1.0 / D)
    ex2 = sb.tile([1, S_T], f32, tag="ln_t", bufs=4, name="ex2")
    nc.scalar.activation(ex2[:], msq_ps[:], AF.Identity, scale=1.0 / D)
    var = sb.tile([1, S_T], f32, tag="ln_t", bufs=4, name="var")
    nc.vector.tensor_tensor(out=var[:], in0=mu[:], in1=mu[:], op=ALU.mult)
    nc.vector.tensor_tensor(out=var[:], in0=ex2[:], in1=var[:], op=ALU.subtract)
    sd = sb.tile([1, S_T], f32, tag="ln_t", bufs=4, name="sd")
    nc.scalar.activation(sd[:], var[:], AF.Sqrt, bias=eps_t[:])
    rstd = sb.tile([1, S_T], f32, tag="ln_r", bufs=2, name="rstd")
    nc.vector.reciprocal(rstd[:], sd[:])
    nmu = sb.tile([1, S_T], f32, tag="ln_r", bufs=2, name="nmu")
    nc.vector.scalar_tensor_tensor(
        out=nmu[:], in0=mu[:], scalar=-1.0, in1=rstd[:],
        op0=ALU.mult, op1=ALU.mult)
    nc.gpsimd.partition_broadcast(scaleb[:], rstd[:])
    nc.gpsimd.partition_broadcast(biasb[:], nmu[:])
    return scaleb, biasb


def _kv_stage(nc, tc, sb, wp, pa, yprev, memsm_dram, wk, wv, n_kv):
    """K/V natural projections + A accumulation for one batch elem.
    Returns a_sb [128, 8, DQ] bf16 (A per head). A matmuls run one sm-tile
    behind the projections so the PE never waits on the evac chain."""
    a_sb = sb.tile([128, 8, DQ], bf16, tag="a_sb", bufs=2)
    a_acc = sb.tile([128, 1024], f32, tag="a_acc", bufs=2)
    pend = []

    # NOTE: psum accumulation groups must not interleave within one bank, so
    # each sm's A-partial is a single-shot matmul set, accumulated on the DVE.
    def emit_a(sm, ek, ev):
        apart = pa.tile([128, 1024], f32, tag="apart", bufs=1)
        for h in range(H):
            nc.tensor.matmul(apart[:, DQ * h:DQ * (h + 1)],
                             ek[:, h, :], ev[:, h, :])
        if sm == 0:
            nc.vector.tensor_copy(a_acc[:], apart[:])
        else:
            nc.vector.tensor_tensor(out=a_acc[:], in0=a_acc[:], in1=apart[:],
                                    op=ALU.add)

    for sm in range(n_kv):
        if memsm_dram is None:
            def lhsT(k, sm=sm):
                return yprev[:, k, DQ * sm:DQ * (sm + 1)]
        else:
            mt = wp.tile([128, 8, DQ], bf16, tag="memsm", bufs=4)
            nc.sync.dma_start(mt[:], memsm_dram[sm])
            def lhsT(k, mt=mt):
                return mt[:, k, :]
        klo = pa.tile([128, 512], f32, tag="kv", bufs=6, name="klo")
        khi = pa.tile([128, 512], f32, tag="kv", bufs=6, name="khi")
        vlo = pa.tile([128, 512], f32, tag="kv", bufs=6, name="vlo")
        vhi = pa.tile([128, 512], f32, tag="kv", bufs=6, name="vhi")
        for k in range(8):
            lt = lhsT(k)
            nc.tensor.matmul(klo[:], lt, wk[:, k, 0:4, :], start=(k == 0), stop=(k == 7))
            nc.tensor.matmul(khi[:], lt, wk[:, k, 4:8, :], start=(k == 0), stop=(k == 7))
            nc.tensor.matmul(vlo[:], lt, wv[:, k, 0:4, :], start=(k == 0), stop=(k == 7))
            nc.tensor.matmul(vhi[:], lt, wv[:, k, 4:8, :], start=(k == 0), stop=(k == 7))
        expk = wp.tile([128, 8, DQ], bf16, tag="expk", bufs=3)
        nc.scalar.activation(expk[:, 0:4, :], klo[:], AF.Exp, scale=1.0 / SCALE)
        nc.scalar.activation(expk[:, 4:8, :], khi[:], AF.Exp, scale=1.0 / SCALE)
        krs = wp.tile([128, 8], f32, tag="krs", bufs=2)
        nc.vector.tensor_reduce(out=krs[:], in_=expk[:],
                                axis=mybir.AxisListType.X, op=ALU.add)
        krr = wp.tile([128, 8], f32, tag="krr", bufs=2)
        nc.vector.reciprocal(krr[:], krs[:])
        expv = wp.tile([128, 8, DQ], bf16, tag="expv", bufs=3)
        nc.vector.tensor_tensor(
            out=expv[:, 0:4, :], in0=vlo[:].rearrange("p (h q) -> p h q", h=4),
            in1=krr[:, 0:4].unsqueeze(2).broadcast_to([128, 4, DQ]), op=ALU.mult)
        nc.vector.tensor_tensor(
            out=expv[:, 4:8, :], in0=vhi[:].rearrange("p (h q) -> p h q", h=4),
            in1=krr[:, 4:8].unsqueeze(2).broadcast_to([128, 4, DQ]), op=ALU.mult)
        pend.append((sm, expk, expv))
        if len(pend) > 1:
            emit_a(*pend.pop(0))
    emit_a(*pend.pop(0))
    nc.vector.tensor_copy(a_sb[:].rearrange("p h q -> p (h q)"), a_acc[:])
    return a_sb


def _q_stage(nc, tc, sb, wp, pq, yprev, a_sb, wq, masked, maskt, ones_bf):
    """Q proj + softmax + BmT for one batch elem -> bmtp [128, 8, S_T].
    Pass-structured so the PE stream never waits on the softmax chain."""
    bmtp = sb.tile([128, 8, S_T], bf16, tag="bmtp", bufs=2)
    expqs = []
    for h in range(H):
        qt = pq.tile([128, S_T], f32, tag="qt", bufs=3)
        for k in range(8):
            nc.tensor.matmul(qt[:, 0:512], wq[:, h, k, :], yprev[:, k, 0:512],
                             start=(k == 0), stop=(k == 7))
            nc.tensor.matmul(qt[:, 512:1024], wq[:, h, k, :], yprev[:, k, 512:1024],
                             start=(k == 0), stop=(k == 7))
        if masked:
            nc.vector.tensor_tensor(out=qt[:, 0:DQ], in0=qt[:, 0:DQ],
                                    in1=maskt[:], op=ALU.add)
        expq = wp.tile([128, S_T], bf16, tag="expq", bufs=8, name=f"expq{h}")
        nc.scalar.activation(expq[:], qt[:], AF.Exp, scale=1.0 / SCALE)
        expqs.append(expq)
    for h in range(H):
        qs = pq.tile([1, S_T], f32, tag="qs", bufs=1)
        nc.tensor.matmul(qs[:, 0:512], ones_bf[:], expqs[h][:, 0:512])
        nc.tensor.matmul(qs[:, 512:1024], ones_bf[:], expqs[h][:, 512:1024])
        qsr = wp.tile([1, S_T], f32, tag="qsr", bufs=2)
        nc.vector.reciprocal(qsr[:], qs[:])
        qsb = wp.tile([128, S_T], f32, tag="qsb", bufs=2)
        nc.gpsimd.partition_broadcast(qsb[:], qsr[:])
        nc.vector.tensor_tensor(out=expqs[h][:], in0=expqs[h][:], in1=qsb[:],
                                op=ALU.mult)
    for h in range(H):
        bmt = pq.tile([128, S_T], f32, tag="qt", bufs=3)
        nc.tensor.matmul(bmt[:, 0:512], a_sb[:, h, :], expqs[h][:, 0:512])
        nc.tensor.matmul(bmt[:, 512:1024], a_sb[:, h, :], expqs[h][:, 512:1024])
        nc.scalar.activation(bmtp[:, h, :], bmt[:], AF.Identity)
    return bmtp


def _out_stage(nc, tc, sb, wp, po, yprev, ynext, bmtp, wo, ones_bf, eps_t,
               rsd, scaleb, biasb, gb=None, gbi=0):
    """Wo matmul (transposed out) + residual + LN for one batch elem.
    Stats matmuls run one cc-tile behind so the PE never waits. rsd and the
    LN scale/bias tiles live in the persistent pool so this phase's pools can
    close (and the next phase start) while the normalize ops drain."""
    bmv = bmtp[:].rearrange("p h (m e) -> p (h m) e", e=8)
    mean_ps = po.tile([1, S_T], f32, tag="mean", bufs=1)
    msq_ps = po.tile([1, S_T], f32, tag="msq", bufs=1)
    pend = []

    def emit_stats(cc, rsq):
        nc.tensor.matmul(mean_ps[:, 0:512], ones_bf[:], rsd[:, cc, 0:512],
                         start=(cc == 0), stop=(cc == 7))
        nc.tensor.matmul(mean_ps[:, 512:1024], ones_bf[:], rsd[:, cc, 512:1024],
                         start=(cc == 0), stop=(cc == 7))
        nc.tensor.matmul(msq_ps[:, 0:512], ones_bf[:], rsq[:, 0:512],
                         start=(cc == 0), stop=(cc == 7))
        nc.tensor.matmul(msq_ps[:, 512:1024], ones_bf[:], rsq[:, 512:1024],
                         start=(cc == 0), stop=(cc == 7))

    for cc in range(8):
        ot = po.tile([128, S_T], f32, tag="ot", bufs=2)
        for j in range(8):
            nc.tensor.matmul(ot[:, 0:512], wo[:, j, cc, :], bmv[:, 0:512, j],
                             start=(j == 0), stop=(j == 7))
            nc.tensor.matmul(ot[:, 512:1024], wo[:, j, cc, :], bmv[:, 512:1024, j],
                             start=(j == 0), stop=(j == 7))
        nc.vector.tensor_tensor(out=rsd[:, cc, :], in0=ot[:], in1=yprev[:, cc, :],
                                op=ALU.add)
        rsq = wp.tile([128, S_T], bf16, tag="rsq", bufs=2)
        nc.scalar.activation(rsq[:], rsd[:, cc, :], AF.Square)
        pend.append((cc, rsq))
        if len(pend) > 1:
            emit_stats(*pend.pop(0))
    emit_stats(*pend.pop(0))
    _ln_tail(nc, wp, mean_ps, msq_ps, eps_t, scaleb, biasb)
    for cc in range(8):
        nc.vector.tensor_tensor(out=ynext[:, cc, :], in0=rsd[:, cc, :],
                                in1=scaleb[:], op=ALU.mult)
        nc.vector.tensor_tensor(out=ynext[:, cc, :], in0=ynext[:, cc, :],
                                in1=biasb[:], op=ALU.add)
        if gb is not None:
            nc.scalar.activation(ynext[:, cc, :], ynext[:, cc, :], AF.Identity,
                                 scale=gb[gbi][:, cc:cc + 1],
                                 bias=gb[gbi + 1][:, cc:cc + 1])


def _lffn_mm(nc, tc, sb, pf, src, dst, wtile, kn, tn, act):
    """dst[:, t, :] = act(sum_k wtile[:,k,t,:].T @ src[:, k, :]) for t<tn."""
    for t in range(tn):
        ps = pf.tile([128, S_T], f32, tag="facc", bufs=3)
        for k in range(kn):
            nc.tensor.matmul(ps[:, 0:512], wtile[:, k, t, :], src[:, k, 0:512],
                             start=(k == 0), stop=(k == kn - 1))
            nc.tensor.matmul(ps[:, 512:1024], wtile[:, k, t, :], src[:, k, 512:1024],
                             start=(k == 0), stop=(k == kn - 1))
        nc.scalar.activation(dst[:, t, :], ps[:], act)


def _build(affine: bool):
    nc = bacc.Bacc("TRN2", target_bir_lowering=False, debug=False,
                   enable_asserts=True, num_devices=N_CORES)

    def din(name, shape, dt=bf16):
        return nc.dram_tensor(name, list(shape), dt, kind="ExternalInput").ap()

    y0T_d = din("y0T", [BPC, 128, 8, S_T])
    memT_d = din("memT", [BPC, 16, 128, 8 * DQ])
    wq1_d = din("wq1", [128, 8, 8, DQ])
    wkv1_d = din("wkv1", [2, 128, 8, 8, DQ])
    wo1_d = din("wo1", [128, 8, 8, DQ])
    wq2_d = din("wq2", [128, 8, 8, DQ])
    wkv2_d = din("wkv2", [2, 128, 8, 8, DQ])
    wo2_d = din("wo2", [128, 8, 8, DQ])
    e1p_d = din("e1p", [128, 8, 4, DQ])
    d1p_d = din("d1p", [128, 4, 8, DQ])
    e2p_d = din("e2p", [128, 8, 4, DQ])
    d2p_d = din("d2p", [128, 4, 8, DQ])
    mask_d = din("maskneg", [128, DQ], f32)
    ones_d = din("ones", [128, 1])
    if affine:
        grep_d = din("grep", [6, 128, 8], f32)

    outT = nc.dram_tensor("outT", [BPC, 128, 8, S_T], f32,
                          kind="ExternalOutput").ap()

    with tile.TileContext(nc) as tc:
        with tc.tile_pool(name="glob", bufs=1) as gl:
            maskt = gl.tile([128, DQ], f32, tag="maskt")
            nc.sync.dma_start(maskt[:], mask_d[:])
            ones_bf = gl.tile([128, 1], bf16, tag="ones")
            nc.sync.dma_start(ones_bf[:], ones_d[:])
            eps_t = gl.tile([1, 1], f32, tag="eps")
            nc.vector.memset(eps_t[:], EPS)
            gb = None
            if affine:
                gb = [gl.tile([128, 8], f32, tag=f"gb{i}", name=f"gb{i}")
                      for i in range(6)]
                for i in range(6):
                    nc.sync.dma_start(gb[i][:], grep_d[i])

            # rotating activation generations per batch elem (bufs=2)
            def ytile(b, g):
                return gl.tile([128, 8, S_T], bf16, tag=f"yT{b}", bufs=2,
                               name=f"yT{b}_{g}")

            y0 = [ytile(b, 0) for b in range(BPC)]
            nc.sync.dma_start(y0[0][:], y0T_d[0])

            # ---- attention phases ----
            # wq loads at phase start (overlaps KV stage); wo loads at Q-stage
            # start (overlaps Q); wk/wv freed before the Q stage runs.
            def attn(yprev, ynext, memsm, wq_d, wkv_d, wo_d, masked, n_kv, gbi,
                     prefetch=()):
                with tc.tile_pool(name="work", bufs=1) as sb, \
                     tc.tile_pool(name="wqp", bufs=1) as wqp:
                    wq = wqp.tile([128, 8, 8, DQ], bf16, tag="wq")
                    nc.sync.dma_start(wq[:], wq_d[:])
                    for dst, srcd in prefetch:
                        nc.sync.dma_start(dst[:], srcd)
                    with tc.tile_pool(name="wkvp", bufs=1) as wkvp:
                        wk = wkvp.tile([128, 8, 8, DQ], bf16, tag="wk")
                        nc.sync.dma_start(wk[:], wkv_d[0])
                        wv = wkvp.tile([128, 8, 8, DQ], bf16, tag="wv")
                        nc.sync.dma_start(wv[:], wkv_d[1])
                        with tc.tile_pool(name="kvt", bufs=1) as kvt:
                            with tc.tile_pool(name="ps_kv", bufs=1, space="PSUM") as pa:
                                a_sbs = [
                                    _kv_stage(nc, tc, sb, kvt, pa, yprev[b],
                                              None if memsm is None else memsm[b],
                                              wk, wv, n_kv)
                                    for b in range(BPC)]
                    with tc.tile_pool(name="wop", bufs=1) as wop:
                        wo = wop.tile([128, 8, 8, DQ], bf16, tag="wo")
                        nc.sync.dma_start(wo[:], wo_d[:])
                        with tc.tile_pool(name="qtp", bufs=1) as qtp:
                            with tc.tile_pool(name="ps_q", bufs=1, space="PSUM") as pq:
                                bmtps = [
                                    _q_stage(nc, tc, sb, qtp, pq, yprev[b], a_sbs[b],
                                             wq, masked, maskt, ones_bf)
                                    for b in range(BPC)]
                        with tc.tile_pool(name="otp", bufs=1) as otp:
                            with tc.tile_pool(name="ps_o", bufs=1, space="PSUM") as po:
                                for b in range(BPC):
                                    rsd = gl.tile([128, 8, S_T], bf16,
                                                  tag="rsd", bufs=1, name="rsd")
                                    scb = gl.tile([128, S_T], f32,
                                                  tag="ln_scaleb", bufs=1, name="scb")
                                    bib = gl.tile([128, S_T], f32,
                                                  tag="ln_biasb", bufs=1, name="bib")
                                    _out_stage(nc, tc, sb, otp, po, yprev[b], ynext[b],
                                               bmtps[b], wo, ones_bf, eps_t,
                                               rsd, scb, bib, gb=gb, gbi=gbi)

            y1 = [ytile(b, 1) for b in range(BPC)]
            attn(y0, y1, None, wq1_d, wkv1_d, wo1_d, True, 8, 0,
                 prefetch=[(y0[b], y0T_d[b]) for b in range(1, BPC)])
            y2 = [ytile(b, 2) for b in range(BPC)]
            attn(y1, y2, memT_d, wq2_d, wkv2_d, wo2_d, False, 16, 2)

            # ---- LFFN ----
            with tc.tile_pool(name="workf", bufs=1) as sb:
                with tc.tile_pool(name="wf", bufs=1) as wp:
                    e1p = wp.tile([128, 8, 4, DQ], bf16, tag="e1p")
                    nc.sync.dma_start(e1p[:], e1p_d[:])
                    d1p = wp.tile([128, 4, 8, DQ], bf16, tag="d1p")
                    nc.sync.dma_start(d1p[:], d1p_d[:])
                    e2p = wp.tile([128, 8, 4, DQ], bf16, tag="e2p")
                    nc.sync.dma_start(e2p[:], e2p_d[:])
                    d2p = wp.tile([128, 4, 8, DQ], bf16, tag="d2p")
                    nc.sync.dma_start(d2p[:], d2p_d[:])
                    for b in range(BPC):
                        h1T = sb.tile([128, 4, S_T], bf16, tag="h1T", bufs=1)
                        swT = sb.tile([128, 8, S_T], bf16, tag="swT", bufs=1)
                        g1T = sb.tile([128, 4, S_T], bf16, tag="g1T", bufs=1)
                        with tc.tile_pool(name="ps_f", bufs=1, space="PSUM") as pf:
                            _lffn_mm(nc, tc, sb, pf, y2[b], h1T, e1p, 8, 4, AF.Identity)
                            _lffn_mm(nc, tc, sb, pf, h1T, swT, d1p, 4, 8, AF.Silu)
                            _lffn_mm(nc, tc, sb, pf, swT, g1T, e2p, 8, 4, AF.Identity)
                        with tc.tile_pool(name="ps_f2", bufs=1, space="PSUM") as po:
                            # D2 matmul + residual + LN3 (writes outf f32)
                            rsd = gl.tile([128, 8, S_T], bf16, tag="rsd",
                                          bufs=1, name="rsd")
                            scb = gl.tile([128, S_T], f32, tag="ln_scaleb",
                                          bufs=1, name="scb")
                            bib = gl.tile([128, S_T], f32, tag="ln_biasb",
                                          bufs=1, name="bib")
                            mean_ps = po.tile([1, S_T], f32, tag="mean", bufs=1)
                            msq_ps = po.tile([1, S_T], f32, tag="msq", bufs=1)
                            def emit_stats(cc, rsq):
                                nc.tensor.matmul(mean_ps[:, 0:512], ones_bf[:],
                                                 rsd[:, cc, 0:512],
                                                 start=(cc == 0), stop=(cc == 7))
                                nc.tensor.matmul(mean_ps[:, 512:1024], ones_bf[:],
                                                 rsd[:, cc, 512:1024],
                                                 start=(cc == 0), stop=(cc == 7))
                                nc.tensor.matmul(msq_ps[:, 0:512], ones_bf[:],
                                                 rsq[:, 0:512],
                                                 start=(cc == 0), stop=(cc == 7))
                                nc.tensor.matmul(msq_ps[:, 512:1024], ones_bf[:],
                                                 rsq[:, 512:1024],
                                                 start=(cc == 0), stop=(cc == 7))
                            pend = []
                            for cc in range(8):
                                ps = po.tile([128, S_T], f32, tag="ot", bufs=2)
                                for k in range(4):
                                    nc.tensor.matmul(
                                        ps[:, 0:512], d2p[:, k, cc, :],
                                        g1T[:, k, 0:512],
                                        start=(k == 0), stop=(k == 3))
                                    nc.tensor.matmul(
                                        ps[:, 512:1024], d2p[:, k, cc, :],
                                        g1T[:, k, 512:1024],
                                        start=(k == 0), stop=(k == 3))
                                nc.vector.tensor_tensor(
                                    out=rsd[:, cc, :], in0=ps[:],
                                    in1=y2[b][:, cc, :], op=ALU.add)
                                rsq = sb.tile([128, S_T], bf16, tag="rsq", bufs=2)
                                nc.scalar.activation(rsq[:], rsd[:, cc, :], AF.Square)
                                pend.append((cc, rsq))
                                if len(pend) > 1:
                                    emit_stats(*pend.pop(0))
                            emit_stats(*pend.pop(0))
                            scaleb, biasb = _ln_tail(nc, sb, mean_ps, msq_ps,
                                                     eps_t, scb, bib)
                            for cc in range(8):
                                outf = sb.tile([128, S_T], f32, tag="outf", bufs=2)
                                nc.vector.tensor_tensor(
                                    out=outf[:], in0=rsd[:, cc, :],
                                    in1=scaleb[:], op=ALU.mult)
                                nc.vector.tensor_tensor(
                                    out=outf[:], in0=outf[:],
                                    in1=biasb[:], op=ALU.add)
                                if gb is not None:
                                    nc.scalar.activation(
                                        outf[:], outf[:], AF.Identity,
                                        scale=gb[4][:, cc:cc + 1],
                                        bias=gb[5][:, cc:cc + 1])
                                nc.sync.dma_start(outT[b][:, cc, :], outf[:])

    nc.compile()
    return nc


_CACHE = {}


def _prep_host(inputs):
    g = {k: np.asarray(v) for k, v in inputs.items()}
    affine = not (
        np.all(g["g1"] == 1) and np.all(g["g2"] == 1) and np.all(g["g3"] == 1)
        and np.all(g["b1"] == 0) and np.all(g["b2"] == 0) and np.all(g["b3"] == 0))

    def wq_pack(w):  # stationary: [128, h, k, dq]
        return np.ascontiguousarray(
            w.transpose(1, 0, 2).reshape(8, 128, 8, DQ).transpose(1, 2, 0, 3)
        ).astype(bf)

    def wkv_pack(wk_, wv_):  # moving: [2, 128, k, h, dq]
        def one(w):
            # w [H, D, DQ]: arr[p, k, h, :] = w[h, 128k+p, :]
            return w.transpose(1, 0, 2).reshape(8, 128, 8, DQ).transpose(1, 0, 2, 3)
        return np.ascontiguousarray(np.stack([one(wk_), one(wv_)])).astype(bf)

    def lhsT_pack(wT, kn, tn):  # [128, k, t, 128] from wT [kn*128, tn*128]
        return np.ascontiguousarray(
            wT.reshape(kn, 128, tn, DQ).transpose(1, 0, 2, 3)).astype(bf)

    host = {}
    host["wq1"] = wq_pack(g["Wq1"])
    host["wkv1"] = wkv_pack(g["Wk1"], g["Wv1"])
    host["wo1"] = lhsT_pack(np.ascontiguousarray(g["Wo1"].T), 8, 8)
    host["wq2"] = wq_pack(g["Wq2"])
    host["wkv2"] = wkv_pack(g["Wk2"], g["Wv2"])
    host["wo2"] = lhsT_pack(np.ascontiguousarray(g["Wo2"].T), 8, 8)
    host["e1p"] = lhsT_pack(np.ascontiguousarray(g["E1"].T), 8, 4)
    host["d1p"] = lhsT_pack(np.ascontiguousarray(g["D1"].T), 4, 8)
    host["e2p"] = lhsT_pack(np.ascontiguousarray(g["E2"].T), 8, 4)
    host["d2p"] = lhsT_pack(np.ascontiguousarray(g["D2"].T), 4, 8)
    host["maskneg"] = np.where(
        np.arange(128)[:, None] <= np.arange(DQ)[None, :], 0.0, NEG
    ).astype(np.float32)
    host["ones"] = np.ones((128, 1), np.float32).astype(bf)
    if affine:
        host["grep"] = np.stack([
            g[n].astype(np.float32).reshape(8, 128).T
            for n in ("g1", "b1", "g2", "b2", "g3", "b3")]).copy()

    in_maps = []
    y = g["y"].astype(np.float32)
    mem = g["mem"].astype(np.float32)
    for c in range(N_CORES):
        sl = slice(BPC * c, BPC * (c + 1))
        m = dict(host)
        # y0T [b][p, k, s]: = y[b, s, 128k+p]
        yT_ = y[sl].transpose(0, 2, 1)  # [b, D, S]
        m["y0T"] = np.ascontiguousarray(
            yT_.reshape(BPC, 8, 128, S_T).transpose(0, 2, 1, 3)).astype(bf)
        # memT [b][sm][p, 128k+sigma] = mem[b, 128sm+sigma, 128k+p]
        mm = mem[sl].reshape(BPC, 16, 128, 8, 128)  # [b, sm, sigma, k, p]
        m["memT"] = np.ascontiguousarray(
            mm.transpose(0, 1, 4, 3, 2).reshape(BPC, 16, 128, 8 * DQ)).astype(bf)
        in_maps.append(m)
    return in_maps, affine


def kernel(**inputs):
    in_maps, affine = _prep_host(inputs)
    if affine not in _CACHE:
        _CACHE[affine] = _build(affine)
    nc = _CACHE[affine]
    res = run_bass_kernel_spmd(nc, in_maps, list(range(N_CORES)))
    # outT [BPC, 128, 8, S_T] -> out[b, s, 128cc+p] = outT[b, p, cc, s]
    outs = []
    for r in res.results:
        o = r["outT"].reshape(BPC, 128, 8, S_T).transpose(0, 3, 2, 1)
        outs.append(np.ascontiguousarray(o.reshape(BPC, S_T, D)))
    return np.concatenate(outs, axis=0)


# revision 6
# speedup vs baseline: 1.1052x; 1.0099x over previous
# Trainium2 Bass kernel for nn_DecoderBlock — transposed-activation design.
#
# Sharding: data-parallel over batch — 16 elems / 8 cores = 2 per core.
#
# All activations live in SBUF in TRANSPOSED layout xT [D(part, 8 chunks), S]
# for the whole kernel; natural layout is never materialized on device (the
# host transposes the final output back, which is free).
#
# Per head h:
#   QT = Wq[h].T @ xT            (lhsT = Wq chunks, rhs = xT)      [dq, S]
#   expQT = exp((QT + mask)/sc)  -> qs = colsum via ones-matmul    [1, S]
#   softQT = expQT * bcast(1/qs) (GPSIMD partition_broadcast)
#   K/V natural per sm-tile: lhsT = xT s-chunk, rhs = Wk/Wv packed [s, 8*dq]
#   expK, V' = V / rowsum(expK); A[h] += expK[h].T @ V'[h]         [dq, dq]
#   BmT[h] = A[h].T @ softQT                                       [dq, S]
# Output (torch .view(b,w,h*d) quirk folded into a strided AP):
#   outT[cc] = sum_j WoT[j,cc].T @ BmTpack[:, w', j]   (w' = 128h+q, s = 8q+j)
# Residual + LayerNorm in transposed layout: stats over the partition axis
# via ones-matmuls, per-column scale/bias replicated with partition_broadcast.
# LFFN fully transposed: h1T = E1 @ yT, h2T = D1 @ h1T, silu, E2, D2.
import numpy as np
import ml_dtypes

import concourse.bacc as bacc
import concourse.mybir as mybir
import concourse.tile as tile
from concourse.bass_utils import run_bass_kernel_spmd

H, D, DQ, BNK, HID = 8, 1024, 128, 512, 1024
B, S_T, S_M = 16, 1024, 2048
SCALE = DQ ** 0.25
EPS = 1e-5
NEG = -200.0
N_CORES = 8
BPC = B // N_CORES

f32 = mybir.dt.float32
bf16 = mybir.dt.bfloat16
AF = mybir.ActivationFunctionType
ALU = mybir.AluOpType
bf = ml_dtypes.bfloat16


def _ln_tail(nc, sb, mean_ps, msq_ps, eps_t, scaleb, biasb):
    """Column stats [1,S] -> broadcast scale/bias into the given tiles."""
    mu = sb.tile([1, S_T], f32, tag="ln_t", bufs=4, name="mu")
    nc.scalar.activation(mu[:], mean_ps[:], AF.Identity, scale=1.0 / D)
    ex2 = sb.tile([1, S_T], f32, tag="ln_t", bufs=4, name="ex2")
    nc.scalar.activation(ex2[:], msq_ps[:], AF.Identity, scale=1.0 / D)
    var = sb.tile([1, S_T], f32, tag="ln_t", bufs=4, name="var")
    nc.vector.tensor_tensor(out=var[:], in0=mu[:], in1=mu[:], op=ALU.mult)
    nc.vector.tensor_tensor(out=var[:], in0=ex2[:], in1=var[:], op=ALU.subtract)
    sd = sb.tile([1, S_T], f32, tag="ln_t", bufs=4, name="sd")
    nc.scalar.activation(sd[:], var[:], AF.Sqrt, bias=eps_t[:])
    rstd = sb.tile([1, S_T], f32, tag="ln_r", bufs=2, name="rstd")
    nc.vector.reciprocal(rstd[:], sd[:])
    nmu = sb.tile([1, S_T], f32, tag="ln_r", bufs=2, name="nmu")
    nc.vector.scalar_tensor_tensor(
        out=nmu[:], in0=mu[:], scalar=-1.0, in1=rstd[:],
        op0=ALU.mult, op1=ALU.mult)
    nc.gpsimd.partition_broadcast(scaleb[:], rstd[:])
    nc.gpsimd.partition_broadcast(biasb[:], nmu[:])
    return scaleb, biasb


def _kv_stage(nc, tc, sb, wp, pa, yprev, memsm_dram, wk, wv, n_kv):
    """K/V natural projections + A accumulation for one batch elem.
    Returns a_sb [128, 8, DQ] bf16 (A per head). A matmuls run one sm-tile
    behind the projections so the PE never waits on the evac chain."""
    a_sb = sb.tile([128, 8, DQ], bf16, tag="a_sb", bufs=2)
    a_acc = sb.tile([128, 1024], f32, tag="a_acc", bufs=2)
    pend = []

    # NOTE: psum accumulation groups must not interleave within one bank, so
    # each sm's A-partial is a single-shot matmul set, accumulated on the DVE.
    def emit_a(sm, ek, ev):
        apart = pa.tile([128, 1024], f32, tag="apart", bufs=1)
        for h in range(H):
            nc.tensor.matmul(apart[:, DQ * h:DQ * (h + 1)],
                             ek[:, h, :], ev[:, h, :])
        if sm == 0:
            nc.vector.tensor_copy(a_acc[:], apart[:])
        else:
            nc.vector.tensor_tensor(out=a_acc[:], in0=a_acc[:], in1=apart[:],
                                    op=ALU.add)

    for sm in range(n_kv):
        if memsm_dram is None:
            def lhsT(k, sm=sm):
                return yprev[:, k, DQ * sm:DQ * (sm + 1)]
        else:
            mt = wp.tile([128, 8, DQ], bf16, tag="memsm", bufs=4)
            nc.sync.dma_start(mt[:], memsm_dram[sm])
            def lhsT(k, mt=mt):
                return mt[:, k, :]
        klo = pa.tile([128, 512], f32, tag="kv", bufs=6, name="klo")
        khi = pa.tile([128, 512], f32, tag="kv", bufs=6, name="khi")
        vlo = pa.tile([128, 512], f32, tag="kv", bufs=6, name="vlo")
        vhi = pa.tile([128, 512], f32, tag="kv", bufs=6, name="vhi")
        for k in range(8):
            lt = lhsT(k)
            nc.tensor.matmul(klo[:], lt, wk[:, k, 0:4, :], start=(k == 0), stop=(k == 7))
            nc.tensor.matmul(khi[:], lt, wk[:, k, 4:8, :], start=(k == 0), stop=(k == 7))
            nc.tensor.matmul(vlo[:], lt, wv[:, k, 0:4, :], start=(k == 0), stop=(k == 7))
            nc.tensor.matmul(vhi[:], lt, wv[:, k, 4:8, :], start=(k == 0), stop=(k == 7))
        expk = wp.tile([128, 8, DQ], bf16, tag="expk", bufs=3)
        nc.scalar.activation(expk[:, 0:4, :], klo[:], AF.Exp, scale=1.0 / SCALE)
        nc.scalar.activation(expk[:, 4:8, :], khi[:], AF.Exp, scale=1.0 / SCALE)
        krs = wp.tile([128, 8], f32, tag="krs", bufs=2)
        nc.vector.tensor_reduce(out=krs[:], in_=expk[:],
                                axis=mybir.AxisListType.X, op=ALU.add)
        krr = wp.tile([128, 8], f32, tag="krr", bufs=2)
        nc.vector.reciprocal(krr[:], krs[:])
        expv = wp.tile([128, 8, DQ], bf16, tag="expv", bufs=3)
        nc.vector.tensor_tensor(
            out=expv[:, 0:4, :], in0=vlo[:].rearrange("p (h q) -> p h q", h=4),
            in1=krr[:, 0:4].unsqueeze(2).broadcast_to([128, 4, DQ]), op=ALU.mult)
        nc.vector.tensor_tensor(
            out=expv[:, 4:8, :], in0=vhi[:].rearrange("p (h q) -> p h q", h=4),
            in1=krr[:, 4:8].unsqueeze(2).broadcast_to([128, 4, DQ]), op=ALU.mult)
        pend.append((sm, expk, expv))
        if len(pend) > 1:
            emit_a(*pend.pop(0))
    emit_a(*pend.pop(0))
    nc.vector.tensor_copy(a_sb[:].rearrange("p h q -> p (h q)"), a_acc[:])
    return a_sb


def _q_stage(nc, tc, sb, wp, pq, yprev, a_sb, wq, masked, maskt, ones_bf):
    """Q proj + softmax + BmT for one batch elem -> bmtp [128, 8, S_T].
    Pass-structured so the PE stream never waits on the softmax chain."""
    bmtp = sb.tile([128, 8, S_T], bf16, tag="bmtp", bufs=2)
    expqs = []
    for h in range(H):
        qt = pq.tile([128, S_T], f32, tag="qt", bufs=3)
        for k in range(8):
            nc.tensor.matmul(qt[:, 0:512], wq[:, h, k, :], yprev[:, k, 0:512],
                             start=(k == 0), stop=(k == 7))
            nc.tensor.matmul(qt[:, 512:1024], wq[:, h, k, :], yprev[:, k, 512:1024],
                             start=(k == 0), stop=(k == 7))
        if masked:
            nc.vector.tensor_tensor(out=qt[:, 0:DQ], in0=qt[:, 0:DQ],
                                    in1=maskt[:], op=ALU.add)
        expq = wp.tile([128, S_T], bf16, tag="expq", bufs=8, name=f"expq{h}")
        nc.scalar.activation(expq[:], qt[:], AF.Exp, scale=1.0 / SCALE)
        expqs.append(expq)
    for h in range(H):
        qs = pq.tile([1, S_T], f32, tag="qs", bufs=1)
        nc.tensor.matmul(qs[:, 0:512], ones_bf[:], expqs[h][:, 0:512])
        nc.tensor.matmul(qs[:, 512:1024], ones_bf[:], expqs[h][:, 512:1024])
        qsr = wp.tile([1, S_T], f32, tag="qsr", bufs=2)
        nc.vector.reciprocal(qsr[:], qs[:])
        qsb = wp.tile([128, S_T], f32, tag="qsb", bufs=2)
        nc.gpsimd.partition_broadcast(qsb[:], qsr[:])
        nc.vector.tensor_tensor(out=expqs[h][:], in0=expqs[h][:], in1=qsb[:],
                                op=ALU.mult)
    for h in range(H):
        bmt = pq.tile([128, S_T], f32, tag="qt", bufs=3)
        nc.tensor.matmul(bmt[:, 0:512], a_sb[:, h, :], expqs[h][:, 0:512])
        nc.tensor.matmul(bmt[:, 512:1024], a_sb[:, h, :], expqs[h][:, 512:1024])
        nc.scalar.activation(bmtp[:, h, :], bmt[:], AF.Identity)
    return bmtp


def _out_stage(nc, tc, sb, wp, po, yprev, ynext, bmtp, wo, ones_bf, eps_t,
               rsd, scaleb, biasb, gb=None, gbi=0):
    """Wo matmul (transposed out) + residual + LN for one batch elem.
    Stats matmuls run one cc-tile behind so the PE never waits. rsd and the
    LN scale/bias tiles live in the persistent pool so this phase's pools can
    close (and the next phase start) while the normalize ops drain."""
    bmv = bmtp[:].rearrange("p h (m e) -> p (h m) e", e=8)
    mean_ps = po.tile([1, S_T], f32, tag="mean", bufs=1)
    msq_ps = po.tile([1, S_T], f32, tag="msq", bufs=1)
    pend = []

    def emit_stats(cc, rsq):
        nc.tensor.matmul(mean_ps[:, 0:512], ones_bf[:], rsd[:, cc, 0:512],
                         start=(cc == 0), stop=(cc == 7))
        nc.tensor.matmul(mean_ps[:, 512:1024], ones_bf[:], rsd[:, cc, 512:1024],
                         start=(cc == 0), stop=(cc == 7))
        nc.tensor.matmul(msq_ps[:, 0:512], ones_bf[:], rsq[:, 0:512],
                         start=(cc == 0), stop=(cc == 7))
        nc.tensor.matmul(msq_ps[:, 512:1024], ones_bf[:], rsq[:, 512:1024],
                         start=(cc == 0), stop=(cc == 7))

    for cc in range(8):
        ot = po.tile([128, S_T], f32, tag="ot", bufs=2)
        for j in range(8):
            nc.tensor.matmul(ot[:, 0:512], wo[:, j, cc, :], bmv[:, 0:512, j],
                             start=(j == 0), stop=(j == 7))
            nc.tensor.matmul(ot[:, 512:1024], wo[:, j, cc, :], bmv[:, 512:1024, j],
                             start=(j == 0), stop=(j == 7))
        nc.vector.tensor_tensor(out=rsd[:, cc, :], in0=ot[:], in1=yprev[:, cc, :],
                                op=ALU.add)
        rsq = wp.tile([128, S_T], bf16, tag="rsq", bufs=2)
        nc.scalar.activation(rsq[:], rsd[:, cc, :], AF.Square)
        pend.append((cc, rsq))
        if len(pend) > 1:
            emit_stats(*pend.pop(0))
    emit_stats(*pend.pop(0))
    _ln_tail(nc, wp, mean_ps, msq_ps, eps_t, scaleb, biasb)
    for cc in range(8):
        nc.vector.tensor_tensor(out=ynext[:, cc, :], in0=rsd[:, cc, :],
                                in1=scaleb[:], op=ALU.mult)
        nc.vector.tensor_tensor(out=ynext[:, cc, :], in0=ynext[:, cc, :],
                                in1=biasb[:], op=ALU.add)
        if gb is not None:
            nc.scalar.activation(ynext[:, cc, :], ynext[:, cc, :], AF.Identity,
                                 scale=gb[gbi][:, cc:cc + 1],
                                 bias=gb[gbi + 1][:, cc:cc + 1])


def _lffn_mm(nc, tc, sb, pf, src, dst, wtile, kn, tn, act):
    """dst[:, t, :] = act(sum_k wtile[:,k,t,:].T @ src[:, k, :]) for t<tn."""
    for t in range(tn):
        ps = pf.tile([128, S_T], f32, tag="facc", bufs=3)
        for k in range(kn):
            nc.tensor.matmul(ps[:, 0:512], wtile[:, k, t, :], src[:, k, 0:512],
                             start=(k == 0), stop=(k == kn - 1))
            nc.tensor.matmul(ps[:, 512:1024], wtile[:, k, t, :], src[:, k, 512:1024],
                             start=(k == 0), stop=(k == kn - 1))
        nc.scalar.activation(dst[:, t, :], ps[:], act)


def _build(affine: bool):
    nc = bacc.Bacc("TRN2", target_bir_lowering=False, debug=False,
                   enable_asserts=True, num_devices=N_CORES)

    def din(name, shape, dt=bf16):
        return nc.dram_tensor(name, list(shape), dt, kind="ExternalInput").ap()

    y0T_d = din("y0T", [BPC, 128, 8, S_T])
    memT_d = din("memT", [BPC, 16, 128, 8 * DQ])
    wq1_d = din("wq1", [128, 8, 8, DQ])
    wkv1_d = din("wkv1", [2, 128, 8, 8, DQ])
    wo1_d = din("wo1", [128, 8, 8, DQ])
    wq2_d = din("wq2", [128, 8, 8, DQ])
    wkv2_d = din("wkv2", [2, 128, 8, 8, DQ])
    wo2_d = din("wo2", [128, 8, 8, DQ])
    e1p_d = din("e1p", [128, 8, 4, DQ])
    d1p_d = din("d1p", [128, 4, 8, DQ])
    e2p_d = din("e2p", [128, 8, 4, DQ])
    d2p_d = din("d2p", [128, 4, 8, DQ])
    mask_d = din("maskneg", [128, DQ], f32)
    ones_d = din("ones", [128, 1])
    if affine:
        grep_d = din("grep", [6, 128, 8], f32)

    outT = nc.dram_tensor("outT", [BPC, 128, 8, S_T], f32,
                          kind="ExternalOutput").ap()

    with tile.TileContext(nc) as tc:
        with tc.tile_pool(name="glob", bufs=1) as gl:
            maskt = gl.tile([128, DQ], f32, tag="maskt")
            nc.sync.dma_start(maskt[:], mask_d[:])
            ones_bf = gl.tile([128, 1], bf16, tag="ones")
            nc.sync.dma_start(ones_bf[:], ones_d[:])
            eps_t = gl.tile([1, 1], f32, tag="eps")
            nc.vector.memset(eps_t[:], EPS)
            gb = None
            if affine:
                gb = [gl.tile([128, 8], f32, tag=f"gb{i}", name=f"gb{i}")
                      for i in range(6)]
                for i in range(6):
                    nc.sync.dma_start(gb[i][:], grep_d[i])

            # rotating activation generations per batch elem (bufs=2)
            def ytile(b, g):
                return gl.tile([128, 8, S_T], bf16, tag=f"yT{b}", bufs=2,
                               name=f"yT{b}_{g}")

            y0 = [ytile(b, 0) for b in range(BPC)]
            nc.sync.dma_start(y0[0][:], y0T_d[0])

            # ---- attention phases ----
            # wq loads at phase start (overlaps KV stage); wo loads at Q-stage
            # start (overlaps Q); wk/wv freed before the Q stage runs.
            def attn(yprev, ynext, memsm, wq_d, wkv_d, wo_d, masked, n_kv, gbi,
                     prefetch=()):
                with tc.tile_pool(name="work", bufs=1) as sb, \
                     tc.tile_pool(name="wqp", bufs=1) as wqp:
                    wq = wqp.tile([128, 8, 8, DQ], bf16, tag="wq")
                    nc.sync.dma_start(wq[:], wq_d[:])
                    for dst, srcd in prefetch:
                        nc.sync.dma_start(dst[:], srcd)
                    with tc.tile_pool(name="wkvp", bufs=1) as wkvp:
                        # per-chunk loads: the first K/V matmul only waits for
                        # its own k-chunk, not the whole 4MB of weights
                        wk = wkvp.tile([128, 8, 8, DQ], bf16, tag="wk")
                        wv = wkvp.tile([128, 8, 8, DQ], bf16, tag="wv")
                        for k in range(8):
                            nc.sync.dma_start(wk[:, k, :, :], wkv_d[0][:, k])
                            nc.sync.dma_start(wv[:, k, :, :], wkv_d[1][:, k])
                        with tc.tile_pool(name="kvt", bufs=1) as kvt:
                            with tc.tile_pool(name="ps_kv", bufs=1, space="PSUM") as pa:
                                a_sbs = [
                                    _kv_stage(nc, tc, sb, kvt, pa, yprev[b],
                                              None if memsm is None else memsm[b],
                                              wk, wv, n_kv)
                                    for b in range(BPC)]
                    with tc.tile_pool(name="wop", bufs=1) as wop:
                        wo = wop.tile([128, 8, 8, DQ], bf16, tag="wo")
                        nc.sync.dma_start(wo[:], wo_d[:])
                        with tc.tile_pool(name="qtp", bufs=1) as qtp:
                            with tc.tile_pool(name="ps_q", bufs=1, space="PSUM") as pq:
                                bmtps = [
                                    _q_stage(nc, tc, sb, qtp, pq, yprev[b], a_sbs[b],
                                             wq, masked, maskt, ones_bf)
                                    for b in range(BPC)]
                        with tc.tile_pool(name="otp", bufs=1) as otp:
                            with tc.tile_pool(name="ps_o", bufs=1, space="PSUM") as po:
                                for b in range(BPC):
                                    rsd = gl.tile([128, 8, S_T], bf16,
                                                  tag="rsd", bufs=1, name="rsd")
                                    scb = gl.tile([128, S_T], f32,
                                                  tag="ln_scaleb", bufs=1, name="scb")
                                    bib = gl.tile([128, S_T], f32,
                                                  tag="ln_biasb", bufs=1, name="bib")
                                    _out_stage(nc, tc, sb, otp, po, yprev[b], ynext[b],
                                               bmtps[b], wo, ones_bf, eps_t,
                                               rsd, scb, bib, gb=gb, gbi=gbi)

            y1 = [ytile(b, 1) for b in range(BPC)]
            attn(y0, y1, None, wq1_d, wkv1_d, wo1_d, True, 8, 0,
                 prefetch=[(y0[b], y0T_d[b]) for b in range(1, BPC)])
            y2 = [ytile(b, 2) for b in range(BPC)]
            attn(y1, y2, memT_d, wq2_d, wkv2_d, wo2_d, False, 16, 2)

            # ---- LFFN ----
            with tc.tile_pool(name="workf", bufs=1) as sb:
                with tc.tile_pool(name="wf", bufs=1) as wp:
                    e1p = wp.tile([128, 8, 4, DQ], bf16, tag="e1p")
                    nc.sync.dma_start(e1p[:], e1p_d[:])
                    d1p = wp.tile([128, 4, 8, DQ], bf16, tag="d1p")
                    nc.sync.dma_start(d1p[:], d1p_d[:])
                    e2p = wp.tile([128, 8, 4, DQ], bf16, tag="e2p")
                    nc.sync.dma_start(e2p[:], e2p_d[:])
                    d2p = wp.tile([128, 4, 8, DQ], bf16, tag="d2p")
                    nc.sync.dma_start(d2p[:], d2p_d[:])
                    for b in range(BPC):
                        h1T = sb.tile([128, 4, S_T], bf16, tag="h1T", bufs=1)
                        swT = sb.tile([128, 8, S_T], bf16, tag="swT", bufs=1)
                        g1T = sb.tile([128, 4, S_T], bf16, tag="g1T", bufs=1)
                        with tc.tile_pool(name="ps_f", bufs=1, space="PSUM") as pf:
                            _lffn_mm(nc, tc, sb, pf, y2[b], h1T, e1p, 8, 4, AF.Identity)
                            _lffn_mm(nc, tc, sb, pf, h1T, swT, d1p, 4, 8, AF.Silu)
                            _lffn_mm(nc, tc, sb, pf, swT, g1T, e2p, 8, 4, AF.Identity)
                        with tc.tile_pool(name="ps_f2", bufs=1, space="PSUM") as po:
                            # D2 matmul + residual + LN3 (writes outf f32)
                            rsd = gl.tile([128, 8, S_T], bf16, tag="rsd",
                                          bufs=1, name="rsd")
                            scb = gl.tile([128, S_T], f32, tag="ln_scaleb",
                                          bufs=1, name="scb")
                            bib = gl.tile([128, S_T], f32, tag="ln_biasb",
                                          bufs=1, name="bib")
                            mean_ps = po.tile([1, S_T], f32, tag="mean", bufs=1)
                            msq_ps = po.tile([1, S_T], f32, tag="msq", bufs=1)
                            def emit_stats(cc, rsq):
                                nc.tensor.matmul(mean_ps[:, 0:512], ones_bf[:],
                                                 rsd[:, cc, 0:512],
                                                 start=(cc == 0), stop=(cc == 7))
                                nc.tensor.matmul(mean_ps[:, 512:1024], ones_bf[:],
                                                 rsd[:, cc, 512:1024],
                                                 start=(cc == 0), stop=(cc == 7))
                                nc.tensor.matmul(msq_ps[:, 0:512], ones_bf[:],
                                                 rsq[:, 0:512],
                                                 start=(cc == 0), stop=(cc == 7))
                                nc.tensor.matmul(msq_ps[:, 512:1024], ones_bf[:],
                                                 rsq[:, 512:1024],
                                                 start=(cc == 0), stop=(cc == 7))
                            pend = []
                            for cc in range(8):
                                ps = po.tile([128, S_T], f32, tag="ot", bufs=2)
                                for k in range(4):
                                    nc.tensor.matmul(
                                        ps[:, 0:512], d2p[:, k, cc, :],
                                        g1T[:, k, 0:512],
                                        start=(k == 0), stop=(k == 3))
                                    nc.tensor.matmul(
                                        ps[:, 512:1024], d2p[:, k, cc, :],
                                        g1T[:, k, 512:1024],
                                        start=(k == 0), stop=(k == 3))
                                nc.vector.tensor_tensor(
                                    out=rsd[:, cc, :], in0=ps[:],
                                    in1=y2[b][:, cc, :], op=ALU.add)
                                rsq = sb.tile([128, S_T], bf16, tag="rsq", bufs=2)
                                nc.scalar.activation(rsq[:], rsd[:, cc, :], AF.Square)
                                pend.append((cc, rsq))
                                if len(pend) > 1:
                                    emit_stats(*pend.pop(0))
                            emit_stats(*pend.pop(0))
                            scaleb, biasb = _ln_tail(nc, sb, mean_ps, msq_ps,
                                                     eps_t, scb, bib)
                            for cc in range(8):
                                outf = sb.tile([128, S_T], f32, tag="outf", bufs=2)
                                nc.vector.tensor_tensor(
                                    out=outf[:], in0=rsd[:, cc, :],
                                    in1=scaleb[:], op=ALU.mult)
                                nc.vector.tensor_tensor(
                                    out=outf[:], in0=outf[:],
                                    in1=biasb[:], op=ALU.add)
                                if gb is not None:
                                    nc.scalar.activation(
                                        outf[:], outf[:], AF.Identity,
                                        scale=gb[4][:, cc:cc + 1],
                                        bias=gb[5][:, cc:cc + 1])
                                nc.sync.dma_start(outT[b][:, cc, :], outf[:])

    nc.compile()
    return nc


_CACHE = {}


def _prep_host(inputs):
    g = {k: np.asarray(v) for k, v in inputs.items()}
    affine = not (
        np.all(g["g1"] == 1) and np.all(g["g2"] == 1) and np.all(g["g3"] == 1)
        and np.all(g["b1"] == 0) and np.all(g["b2"] == 0) and np.all(g["b3"] == 0))

    def wq_pack(w):  # stationary: [128, h, k, dq]
        return np.ascontiguousarray(
            w.transpose(1, 0, 2).reshape(8, 128, 8, DQ).transpose(1, 2, 0, 3)
        ).astype(bf)

    def wkv_pack(wk_, wv_):  # moving: [2, 128, k, h, dq]
        def one(w):
            # w [H, D, DQ]: arr[p, k, h, :] = w[h, 128k+p, :]
            return w.transpose(1, 0, 2).reshape(8, 128, 8, DQ).transpose(1, 0, 2, 3)
        return np.ascontiguousarray(np.stack([one(wk_), one(wv_)])).astype(bf)

    def lhsT_pack(wT, kn, tn):  # [128, k, t, 128] from wT [kn*128, tn*128]
        return np.ascontiguousarray(
            wT.reshape(kn, 128, tn, DQ).transpose(1, 0, 2, 3)).astype(bf)

    host = {}
    host["wq1"] = wq_pack(g["Wq1"])
    host["wkv1"] = wkv_pack(g["Wk1"], g["Wv1"])
    host["wo1"] = lhsT_pack(np.ascontiguousarray(g["Wo1"].T), 8, 8)
    host["wq2"] = wq_pack(g["Wq2"])
    host["wkv2"] = wkv_pack(g["Wk2"], g["Wv2"])
    host["wo2"] = lhsT_pack(np.ascontiguousarray(g["Wo2"].T), 8, 8)
    host["e1p"] = lhsT_pack(np.ascontiguousarray(g["E1"].T), 8, 4)
    host["d1p"] = lhsT_pack(np.ascontiguousarray(g["D1"].T), 4, 8)
    host["e2p"] = lhsT_pack(np.ascontiguousarray(g["E2"].T), 8, 4)
    host["d2p"] = lhsT_pack(np.ascontiguousarray(g["D2"].T), 4, 8)
    host["maskneg"] = np.where(
        np.arange(128)[:, None] <= np.arange(DQ)[None, :], 0.0, NEG
    ).astype(np.float32)
    host["ones"] = np.ones((128, 1), np.float32).astype(bf)
    if affine:
        host["grep"] = np.stack([
            g[n].astype(np.float32).reshape(8, 128).T
            for n in ("g1", "b1", "g2", "b2", "g3", "b3")]).copy()

    in_maps = []
    y = g["y"].astype(np.float32)
    mem = g["mem"].astype(np.float32)
    for c in range(N_CORES):
        sl = slice(BPC * c, BPC * (c + 1))
        m = dict(host)
        # y0T [b][p, k, s]: = y[b, s, 128k+p]
        yT_ = y[sl].transpose(0, 2, 1)  # [b, D, S]
        m["y0T"] = np.ascontiguousarray(
            yT_.reshape(BPC, 8, 128, S_T).transpose(0, 2, 1, 3)).astype(bf)
        # memT [b][sm][p, 128k+sigma] = mem[b, 128sm+sigma, 128k+p]
        mm = mem[sl].reshape(BPC, 16, 128, 8, 128)  # [b, sm, sigma, k, p]
        m["memT"] = np.ascontiguousarray(
            mm.transpose(0, 1, 4, 3, 2).reshape(BPC, 16, 128, 8 * DQ)).astype(bf)
        in_maps.append(m)
    return in_maps, affine


def kernel(**inputs):
    in_maps, affine = _prep_host(inputs)
    if affine not in _CACHE:
        _CACHE[affine] = _build(affine)
    nc = _CACHE[affine]
    res = run_bass_kernel_spmd(nc, in_maps, list(range(N_CORES)))
    # outT [BPC, 128, 8, S_T] -> out[b, s, 128cc+p] = outT[b, p, cc, s]
    outs = []
    for r in res.results:
        o = r["outT"].reshape(BPC, 128, 8, S_T).transpose(0, 3, 2, 1)
        outs.append(np.ascontiguousarray(o.reshape(BPC, S_T, D)))
    return np.concatenate(outs, axis=0)


# revision 7
# speedup vs baseline: 1.1275x; 1.0201x over previous
# Trainium2 Bass kernel for nn_DecoderBlock — transposed-activation design.
#
# Sharding: data-parallel over batch — 16 elems / 8 cores = 2 per core.
#
# All activations live in SBUF in TRANSPOSED layout xT [D(part, 8 chunks), S]
# for the whole kernel; natural layout is never materialized on device (the
# host transposes the final output back, which is free).
#
# Per head h:
#   QT = Wq[h].T @ xT            (lhsT = Wq chunks, rhs = xT)      [dq, S]
#   expQT = exp((QT + mask)/sc)  -> qs = colsum via ones-matmul    [1, S]
#   softQT = expQT * bcast(1/qs) (GPSIMD partition_broadcast)
#   K/V natural per sm-tile: lhsT = xT s-chunk, rhs = Wk/Wv packed [s, 8*dq]
#   expK, V' = V / rowsum(expK); A[h] += expK[h].T @ V'[h]         [dq, dq]
#   BmT[h] = A[h].T @ softQT                                       [dq, S]
# Output (torch .view(b,w,h*d) quirk folded into a strided AP):
#   outT[cc] = sum_j WoT[j,cc].T @ BmTpack[:, w', j]   (w' = 128h+q, s = 8q+j)
# Residual + LayerNorm in transposed layout: stats over the partition axis
# via ones-matmuls, per-column scale/bias replicated with partition_broadcast.
# LFFN fully transposed: h1T = E1 @ yT, h2T = D1 @ h1T, silu, E2, D2.
import numpy as np
import ml_dtypes

import concourse.bacc as bacc
import concourse.mybir as mybir
import concourse.tile as tile
from concourse.bass_utils import run_bass_kernel_spmd

H, D, DQ, BNK, HID = 8, 1024, 128, 512, 1024
B, S_T, S_M = 16, 1024, 2048
SCALE = DQ ** 0.25
EPS = 1e-5
NEG = -200.0
N_CORES = 8
BPC = B // N_CORES

f32 = mybir.dt.float32
bf16 = mybir.dt.bfloat16
AF = mybir.ActivationFunctionType
ALU = mybir.AluOpType
bf = ml_dtypes.bfloat16


def _ln_tail(nc, sb, mean_ps, msq_ps, eps_t, scaleb, biasb):
    """Column stats [1,S] -> broadcast scale/bias into the given tiles."""
    mu = sb.tile([1, S_T], f32, tag="ln_t", bufs=4, name="mu")
    nc.scalar.activation(mu[:], mean_ps[:], AF.Identity, scale=1.0 / D)
    ex2 = sb.tile([1, S_T], f32, tag="ln_t", bufs=4, name="ex2")
    nc.scalar.activation(ex2[:], msq_ps[:], AF.Identity, scale=1.0 / D)
    var = sb.tile([1, S_T], f32, tag="ln_t", bufs=4, name="var")
    nc.vector.tensor_tensor(out=var[:], in0=mu[:], in1=mu[:], op=ALU.mult)
    nc.vector.tensor_tensor(out=var[:], in0=ex2[:], in1=var[:], op=ALU.subtract)
    sd = sb.tile([1, S_T], f32, tag="ln_t", bufs=4, name="sd")
    nc.scalar.activation(sd[:], var[:], AF.Sqrt, bias=eps_t[:])
    rstd = sb.tile([1, S_T], f32, tag="ln_r", bufs=2, name="rstd")
    nc.vector.reciprocal(rstd[:], sd[:])
    nmu = sb.tile([1, S_T], f32, tag="ln_r", bufs=2, name="nmu")
    nc.vector.scalar_tensor_tensor(
        out=nmu[:], in0=mu[:], scalar=-1.0, in1=rstd[:],
        op0=ALU.mult, op1=ALU.mult)
    nc.gpsimd.partition_broadcast(scaleb[:], rstd[:])
    nc.gpsimd.partition_broadcast(biasb[:], nmu[:])
    return scaleb, biasb


def _kv_stage(nc, tc, sb, wp, pa, yprev, memsm_dram, wk, wv, n_kv):
    """K/V natural projections + A accumulation for one batch elem.
    Returns a_sb [128, 8, DQ] bf16 (A per head). A matmuls run one sm-tile
    behind the projections so the PE never waits on the evac chain."""
    a_sb = sb.tile([128, 8, DQ], bf16, tag="a_sb", bufs=2)
    a_acc = sb.tile([128, 1024], f32, tag="a_acc", bufs=2)
    pend = []

    # NOTE: psum accumulation groups must not interleave within one bank, so
    # each sm's A-partial is a single-shot matmul set, accumulated on the DVE.
    def emit_a(sm, ek, ev):
        apart = pa.tile([128, 1024], f32, tag="apart", bufs=1)
        for h in range(H):
            nc.tensor.matmul(apart[:, DQ * h:DQ * (h + 1)],
                             ek[:, h, :], ev[:, h, :])
        if sm == 0:
            nc.vector.tensor_copy(a_acc[:], apart[:])
        else:
            nc.vector.tensor_tensor(out=a_acc[:], in0=a_acc[:], in1=apart[:],
                                    op=ALU.add)

    for sm in range(n_kv):
        if memsm_dram is None:
            def lhsT(k, sm=sm):
                return yprev[:, k, DQ * sm:DQ * (sm + 1)]
        else:
            mt = wp.tile([128, 8, DQ], bf16, tag="memsm", bufs=4)
            nc.sync.dma_start(mt[:], memsm_dram[sm])
            def lhsT(k, mt=mt):
                return mt[:, k, :]
        klo = pa.tile([128, 512], f32, tag="kv", bufs=6, name="klo")
        khi = pa.tile([128, 512], f32, tag="kv", bufs=6, name="khi")
        vlo = pa.tile([128, 512], f32, tag="kv", bufs=6, name="vlo")
        vhi = pa.tile([128, 512], f32, tag="kv", bufs=6, name="vhi")
        for k in range(8):
            lt = lhsT(k)
            nc.tensor.matmul(klo[:], lt, wk[:, k, 0:4, :], start=(k == 0), stop=(k == 7))
            nc.tensor.matmul(khi[:], lt, wk[:, k, 4:8, :], start=(k == 0), stop=(k == 7))
            nc.tensor.matmul(vlo[:], lt, wv[:, k, 0:4, :], start=(k == 0), stop=(k == 7))
            nc.tensor.matmul(vhi[:], lt, wv[:, k, 4:8, :], start=(k == 0), stop=(k == 7))
        expk = wp.tile([128, 8, DQ], bf16, tag="expk", bufs=3)
        nc.scalar.activation(expk[:, 0:4, :], klo[:], AF.Exp, scale=1.0 / SCALE)
        nc.scalar.activation(expk[:, 4:8, :], khi[:], AF.Exp, scale=1.0 / SCALE)
        krs = wp.tile([128, 8], f32, tag="krs", bufs=2)
        nc.vector.tensor_reduce(out=krs[:], in_=expk[:],
                                axis=mybir.AxisListType.X, op=ALU.add)
        krr = wp.tile([128, 8], f32, tag="krr", bufs=2)
        nc.vector.reciprocal(krr[:], krs[:])
        expv = wp.tile([128, 8, DQ], bf16, tag="expv", bufs=3)
        nc.vector.tensor_tensor(
            out=expv[:, 0:4, :], in0=vlo[:].rearrange("p (h q) -> p h q", h=4),
            in1=krr[:, 0:4].unsqueeze(2).broadcast_to([128, 4, DQ]), op=ALU.mult)
        nc.vector.tensor_tensor(
            out=expv[:, 4:8, :], in0=vhi[:].rearrange("p (h q) -> p h q", h=4),
            in1=krr[:, 4:8].unsqueeze(2).broadcast_to([128, 4, DQ]), op=ALU.mult)
        pend.append((sm, expk, expv))
        if len(pend) > 1:
            emit_a(*pend.pop(0))
    emit_a(*pend.pop(0))
    nc.vector.tensor_copy(a_sb[:].rearrange("p h q -> p (h q)"), a_acc[:])
    return a_sb


def _q_stage(nc, tc, sb, wp, pq, yprev, a_sb, wq, masked, maskt, ones128):
    """Q proj + softmax + BmT for one batch elem -> bmtp [128, 8, S_T].
    Pass-structured so the PE stream never waits on the softmax chain. The
    softmax denominator is computed already replicated across partitions by
    a ones[128,128] matmul, so no partition_broadcast hop is needed."""
    bmtp = sb.tile([128, 8, S_T], bf16, tag="bmtp", bufs=2)
    expqs = []
    for h in range(H):
        qt = pq.tile([128, S_T], f32, tag="qt", bufs=4)
        for k in range(8):
            nc.tensor.matmul(qt[:, 0:512], wq[:, h, k, :], yprev[:, k, 0:512],
                             start=(k == 0), stop=(k == 7))
            nc.tensor.matmul(qt[:, 512:1024], wq[:, h, k, :], yprev[:, k, 512:1024],
                             start=(k == 0), stop=(k == 7))
        if masked:
            nc.vector.tensor_tensor(out=qt[:, 0:DQ], in0=qt[:, 0:DQ],
                                    in1=maskt[:], op=ALU.add)
        expq = wp.tile([128, S_T], bf16, tag="expq", bufs=8, name=f"expq{h}")
        nc.scalar.activation(expq[:], qt[:], AF.Exp, scale=1.0 / SCALE)
        expqs.append(expq)
    for h in range(H):
        qrep = pq.tile([128, S_T], f32, tag="qt", bufs=4)
        nc.tensor.matmul(qrep[:, 0:512], ones128[:], expqs[h][:, 0:512])
        nc.tensor.matmul(qrep[:, 512:1024], ones128[:], expqs[h][:, 512:1024])
        qsb = wp.tile([128, S_T], f32, tag="qsb", bufs=2)
        nc.vector.reciprocal(qsb[:], qrep[:])
        nc.vector.tensor_tensor(out=expqs[h][:], in0=expqs[h][:], in1=qsb[:],
                                op=ALU.mult)
    for h in range(H):
        bmt = pq.tile([128, S_T], f32, tag="qt", bufs=4)
        nc.tensor.matmul(bmt[:, 0:512], a_sb[:, h, :], expqs[h][:, 0:512])
        nc.tensor.matmul(bmt[:, 512:1024], a_sb[:, h, :], expqs[h][:, 512:1024])
        nc.scalar.activation(bmtp[:, h, :], bmt[:], AF.Identity)
    return bmtp


def _out_stage(nc, tc, sb, wp, po, yprev, ynext, bmtp, wo, ones_bf, eps_t,
               rsd, scaleb, biasb, gb=None, gbi=0):
    """Wo matmul (transposed out) + residual + LN for one batch elem.
    Stats matmuls run one cc-tile behind so the PE never waits. rsd and the
    LN scale/bias tiles live in the persistent pool so this phase's pools can
    close (and the next phase start) while the normalize ops drain."""
    bmv = bmtp[:].rearrange("p h (m e) -> p (h m) e", e=8)
    mean_ps = po.tile([1, S_T], f32, tag="mean", bufs=1)
    msq_ps = po.tile([1, S_T], f32, tag="msq", bufs=1)
    pend = []

    def emit_stats(cc, rsq):
        nc.tensor.matmul(mean_ps[:, 0:512], ones_bf[:], rsd[:, cc, 0:512],
                         start=(cc == 0), stop=(cc == 7))
        nc.tensor.matmul(mean_ps[:, 512:1024], ones_bf[:], rsd[:, cc, 512:1024],
                         start=(cc == 0), stop=(cc == 7))
        nc.tensor.matmul(msq_ps[:, 0:512], ones_bf[:], rsq[:, 0:512],
                         start=(cc == 0), stop=(cc == 7))
        nc.tensor.matmul(msq_ps[:, 512:1024], ones_bf[:], rsq[:, 512:1024],
                         start=(cc == 0), stop=(cc == 7))

    for cc in range(8):
        ot = po.tile([128, S_T], f32, tag="ot", bufs=2)
        for j in range(8):
            nc.tensor.matmul(ot[:, 0:512], wo[:, j, cc, :], bmv[:, 0:512, j],
                             start=(j == 0), stop=(j == 7))
            nc.tensor.matmul(ot[:, 512:1024], wo[:, j, cc, :], bmv[:, 512:1024, j],
                             start=(j == 0), stop=(j == 7))
        nc.vector.tensor_tensor(out=rsd[:, cc, :], in0=ot[:], in1=yprev[:, cc, :],
                                op=ALU.add)
        rsq = wp.tile([128, S_T], bf16, tag="rsq", bufs=2)
        nc.scalar.activation(rsq[:], rsd[:, cc, :], AF.Square)
        pend.append((cc, rsq))
        if len(pend) > 1:
            emit_stats(*pend.pop(0))
    emit_stats(*pend.pop(0))
    _ln_tail(nc, wp, mean_ps, msq_ps, eps_t, scaleb, biasb)
    for cc in range(8):
        nc.vector.tensor_tensor(out=ynext[:, cc, :], in0=rsd[:, cc, :],
                                in1=scaleb[:], op=ALU.mult)
        nc.vector.tensor_tensor(out=ynext[:, cc, :], in0=ynext[:, cc, :],
                                in1=biasb[:], op=ALU.add)
        if gb is not None:
            nc.scalar.activation(ynext[:, cc, :], ynext[:, cc, :], AF.Identity,
                                 scale=gb[gbi][:, cc:cc + 1],
                                 bias=gb[gbi + 1][:, cc:cc + 1])


def _lffn_mm(nc, tc, sb, pf, src, dst, wtile, kn, tn, act):
    """dst[:, t, :] = act(sum_k wtile[:,k,t,:].T @ src[:, k, :]) for t<tn."""
    for t in range(tn):
        ps = pf.tile([128, S_T], f32, tag="facc", bufs=3)
        for k in range(kn):
            nc.tensor.matmul(ps[:, 0:512], wtile[:, k, t, :], src[:, k, 0:512],
                             start=(k == 0), stop=(k == kn - 1))
            nc.tensor.matmul(ps[:, 512:1024], wtile[:, k, t, :], src[:, k, 512:1024],
                             start=(k == 0), stop=(k == kn - 1))
        nc.scalar.activation(dst[:, t, :], ps[:], act)


def _build(affine: bool):
    nc = bacc.Bacc("TRN2", target_bir_lowering=False, debug=False,
                   enable_asserts=True, num_devices=N_CORES)

    def din(name, shape, dt=bf16):
        return nc.dram_tensor(name, list(shape), dt, kind="ExternalInput").ap()

    y0T_d = din("y0T", [BPC, 128, 8, S_T])
    memT_d = din("memT", [BPC, 16, 128, 8 * DQ])
    wq1_d = din("wq1", [128, 8, 8, DQ])
    wkv1_d = din("wkv1", [2, 128, 8, 8, DQ])
    wo1_d = din("wo1", [128, 8, 8, DQ])
    wq2_d = din("wq2", [128, 8, 8, DQ])
    wkv2_d = din("wkv2", [2, 128, 8, 8, DQ])
    wo2_d = din("wo2", [128, 8, 8, DQ])
    e1p_d = din("e1p", [128, 8, 4, DQ])
    d1p_d = din("d1p", [128, 4, 8, DQ])
    e2p_d = din("e2p", [128, 8, 4, DQ])
    d2p_d = din("d2p", [128, 4, 8, DQ])
    mask_d = din("maskneg", [128, DQ], f32)
    ones_d = din("ones", [128, 1])
    ones128_d = din("ones128", [128, DQ])
    if affine:
        grep_d = din("grep", [6, 128, 8], f32)

    outT = nc.dram_tensor("outT", [BPC, 128, 8, S_T], f32,
                          kind="ExternalOutput").ap()

    with tile.TileContext(nc) as tc:
        with tc.tile_pool(name="glob", bufs=1) as gl:
            maskt = gl.tile([128, DQ], f32, tag="maskt")
            nc.sync.dma_start(maskt[:], mask_d[:])
            ones_bf = gl.tile([128, 1], bf16, tag="ones")
            nc.sync.dma_start(ones_bf[:], ones_d[:])
            ones128 = gl.tile([128, DQ], bf16, tag="ones128")
            nc.sync.dma_start(ones128[:], ones128_d[:])
            eps_t = gl.tile([1, 1], f32, tag="eps")
            nc.vector.memset(eps_t[:], EPS)
            gb = None
            if affine:
                gb = [gl.tile([128, 8], f32, tag=f"gb{i}", name=f"gb{i}")
                      for i in range(6)]
                for i in range(6):
                    nc.sync.dma_start(gb[i][:], grep_d[i])

            # rotating activation generations per batch elem (bufs=2)
            def ytile(b, g):
                return gl.tile([128, 8, S_T], bf16, tag=f"yT{b}", bufs=2,
                               name=f"yT{b}_{g}")

            y0 = [ytile(b, 0) for b in range(BPC)]
            nc.sync.dma_start(y0[0][:], y0T_d[0])

            # ---- attention phases ----
            # wq loads at phase start (overlaps KV stage); wo loads at Q-stage
            # start (overlaps Q); wk/wv freed before the Q stage runs.
            def attn(yprev, ynext, memsm, wq_d, wkv_d, wo_d, masked, n_kv, gbi,
                     prefetch=()):
                with tc.tile_pool(name="work", bufs=1) as sb, \
                     tc.tile_pool(name="wqp", bufs=1) as wqp:
                    wq = wqp.tile([128, 8, 8, DQ], bf16, tag="wq")
                    nc.sync.dma_start(wq[:], wq_d[:])
                    for dst, srcd in prefetch:
                        nc.sync.dma_start(dst[:], srcd)
                    with tc.tile_pool(name="wkvp", bufs=1) as wkvp:
                        # per-chunk loads: the first K/V matmul only waits for
                        # its own k-chunk, not the whole 4MB of weights
                        wk = wkvp.tile([128, 8, 8, DQ], bf16, tag="wk")
                        wv = wkvp.tile([128, 8, 8, DQ], bf16, tag="wv")
                        for k in range(8):
                            nc.sync.dma_start(wk[:, k, :, :], wkv_d[0][:, k])
                            nc.sync.dma_start(wv[:, k, :, :], wkv_d[1][:, k])
                        with tc.tile_pool(name="kvt", bufs=1) as kvt:
                            with tc.tile_pool(name="ps_kv", bufs=1, space="PSUM") as pa:
                                a_sbs = [
                                    _kv_stage(nc, tc, sb, kvt, pa, yprev[b],
                                              None if memsm is None else memsm[b],
                                              wk, wv, n_kv)
                                    for b in range(BPC)]
                    with tc.tile_pool(name="wop", bufs=1) as wop:
                        wo = wop.tile([128, 8, 8, DQ], bf16, tag="wo")
                        nc.sync.dma_start(wo[:], wo_d[:])
                        with tc.tile_pool(name="qtp", bufs=1) as qtp:
                            with tc.tile_pool(name="ps_q", bufs=1, space="PSUM") as pq:
                                bmtps = [
                                    _q_stage(nc, tc, sb, qtp, pq, yprev[b], a_sbs[b],
                                             wq, masked, maskt, ones128)
                                    for b in range(BPC)]
                        with tc.tile_pool(name="otp", bufs=1) as otp:
                            with tc.tile_pool(name="ps_o", bufs=1, space="PSUM") as po:
                                for b in range(BPC):
                                    rsd = gl.tile([128, 8, S_T], bf16,
                                                  tag="rsd", bufs=1, name="rsd")
                                    scb = gl.tile([128, S_T], f32,
                                                  tag="ln_scaleb", bufs=1, name="scb")
                                    bib = gl.tile([128, S_T], f32,
                                                  tag="ln_biasb", bufs=1, name="bib")
                                    _out_stage(nc, tc, sb, otp, po, yprev[b], ynext[b],
                                               bmtps[b], wo, ones_bf, eps_t,
                                               rsd, scb, bib, gb=gb, gbi=gbi)

            y1 = [ytile(b, 1) for b in range(BPC)]
            attn(y0, y1, None, wq1_d, wkv1_d, wo1_d, True, 8, 0,
                 prefetch=[(y0[b], y0T_d[b]) for b in range(1, BPC)])
            y2 = [ytile(b, 2) for b in range(BPC)]
            attn(y1, y2, memT_d, wq2_d, wkv2_d, wo2_d, False, 16, 2)

            # ---- LFFN ----
            with tc.tile_pool(name="workf", bufs=1) as sb:
                with tc.tile_pool(name="wf", bufs=1) as wp:
                    e1p = wp.tile([128, 8, 4, DQ], bf16, tag="e1p")
                    nc.sync.dma_start(e1p[:], e1p_d[:])
                    d1p = wp.tile([128, 4, 8, DQ], bf16, tag="d1p")
                    nc.sync.dma_start(d1p[:], d1p_d[:])
                    e2p = wp.tile([128, 8, 4, DQ], bf16, tag="e2p")
                    nc.sync.dma_start(e2p[:], e2p_d[:])
                    d2p = wp.tile([128, 4, 8, DQ], bf16, tag="d2p")
                    nc.sync.dma_start(d2p[:], d2p_d[:])
                    for b in range(BPC):
                        h1T = sb.tile([128, 4, S_T], bf16, tag="h1T", bufs=1)
                        swT = sb.tile([128, 8, S_T], bf16, tag="swT", bufs=1)
                        g1T = sb.tile([128, 4, S_T], bf16, tag="g1T", bufs=1)
                        with tc.tile_pool(name="ps_f", bufs=1, space="PSUM") as pf:
                            _lffn_mm(nc, tc, sb, pf, y2[b], h1T, e1p, 8, 4, AF.Identity)
                            _lffn_mm(nc, tc, sb, pf, h1T, swT, d1p, 4, 8, AF.Silu)
                            _lffn_mm(nc, tc, sb, pf, swT, g1T, e2p, 8, 4, AF.Identity)
                        with tc.tile_pool(name="ps_f2", bufs=1, space="PSUM") as po:
                            # D2 matmul + residual + LN3 (writes outf f32)
                            rsd = gl.tile([128, 8, S_T], bf16, tag="rsd",
                                          bufs=1, name="rsd")
                            scb = gl.tile([128, S_T], f32, tag="ln_scaleb",
                                          bufs=1, name="scb")
                            bib = gl.tile([128, S_T], f32, tag="ln_biasb",
                                          bufs=1, name="bib")
                            mean_ps = po.tile([1, S_T], f32, tag="mean", bufs=1)
                            msq_ps = po.tile([1, S_T], f32, tag="msq", bufs=1)
                            def emit_stats(cc, rsq):
                                nc.tensor.matmul(mean_ps[:, 0:512], ones_bf[:],
                                                 rsd[:, cc, 0:512],
                                                 start=(cc == 0), stop=(cc == 7))
                                nc.tensor.matmul(mean_ps[:, 512:1024], ones_bf[:],
                                                 rsd[:, cc, 512:1024],
                                                 start=(cc == 0), stop=(cc == 7))
                                nc.tensor.matmul(msq_ps[:, 0:512], ones_bf[:],
                                                 rsq[:, 0:512],
                                                 start=(cc == 0), stop=(cc == 7))
                                nc.tensor.matmul(msq_ps[:, 512:1024], ones_bf[:],
                                                 rsq[:, 512:1024],
                                                 start=(cc == 0), stop=(cc == 7))
                            pend = []
                            for cc in range(8):
                                ps = po.tile([128, S_T], f32, tag="ot", bufs=2)
                                for k in range(4):
                                    nc.tensor.matmul(
                                        ps[:, 0:512], d2p[:, k, cc, :],
                                        g1T[:, k, 0:512],
                                        start=(k == 0), stop=(k == 3))
                                    nc.tensor.matmul(
                                        ps[:, 512:1024], d2p[:, k, cc, :],
                                        g1T[:, k, 512:1024],
                                        start=(k == 0), stop=(k == 3))
                                nc.vector.tensor_tensor(
                                    out=rsd[:, cc, :], in0=ps[:],
                                    in1=y2[b][:, cc, :], op=ALU.add)
                                rsq = sb.tile([128, S_T], bf16, tag="rsq", bufs=2)
                                nc.scalar.activation(rsq[:], rsd[:, cc, :], AF.Square)
                                pend.append((cc, rsq))
                                if len(pend) > 1:
                                    emit_stats(*pend.pop(0))
                            emit_stats(*pend.pop(0))
                            scaleb, biasb = _ln_tail(nc, sb, mean_ps, msq_ps,
                                                     eps_t, scb, bib)
                            for cc in range(8):
                                outf = sb.tile([128, S_T], f32, tag="outf", bufs=2)
                                nc.vector.tensor_tensor(
                                    out=outf[:], in0=rsd[:, cc, :],
                                    in1=scaleb[:], op=ALU.mult)
                                nc.vector.tensor_tensor(
                                    out=outf[:], in0=outf[:],
                                    in1=biasb[:], op=ALU.add)
                                if gb is not None:
                                    nc.scalar.activation(
                                        outf[:], outf[:], AF.Identity,
                                        scale=gb[4][:, cc:cc + 1],
                                        bias=gb[5][:, cc:cc + 1])
                                nc.sync.dma_start(outT[b][:, cc, :], outf[:])

    nc.compile()
    return nc


_CACHE = {}


def _prep_host(inputs):
    g = {k: np.asarray(v) for k, v in inputs.items()}
    affine = not (
        np.all(g["g1"] == 1) and np.all(g["g2"] == 1) and np.all(g["g3"] == 1)
        and np.all(g["b1"] == 0) and np.all(g["b2"] == 0) and np.all(g["b3"] == 0))

    def wq_pack(w):  # stationary: [128, h, k, dq]
        return np.ascontiguousarray(
            w.transpose(1, 0, 2).reshape(8, 128, 8, DQ).transpose(1, 2, 0, 3)
        ).astype(bf)

    def wkv_pack(wk_, wv_):  # moving: [2, 128, k, h, dq]
        def one(w):
            # w [H, D, DQ]: arr[p, k, h, :] = w[h, 128k+p, :]
            return w.transpose(1, 0, 2).reshape(8, 128, 8, DQ).transpose(1, 0, 2, 3)
        return np.ascontiguousarray(np.stack([one(wk_), one(wv_)])).astype(bf)

    def lhsT_pack(wT, kn, tn):  # [128, k, t, 128] from wT [kn*128, tn*128]
        return np.ascontiguousarray(
            wT.reshape(kn, 128, tn, DQ).transpose(1, 0, 2, 3)).astype(bf)

    host = {}
    host["wq1"] = wq_pack(g["Wq1"])
    host["wkv1"] = wkv_pack(g["Wk1"], g["Wv1"])
    host["wo1"] = lhsT_pack(np.ascontiguousarray(g["Wo1"].T), 8, 8)
    host["wq2"] = wq_pack(g["Wq2"])
    host["wkv2"] = wkv_pack(g["Wk2"], g["Wv2"])
    host["wo2"] = lhsT_pack(np.ascontiguousarray(g["Wo2"].T), 8, 8)
    host["e1p"] = lhsT_pack(np.ascontiguousarray(g["E1"].T), 8, 4)
    host["d1p"] = lhsT_pack(np.ascontiguousarray(g["D1"].T), 4, 8)
    host["e2p"] = lhsT_pack(np.ascontiguousarray(g["E2"].T), 8, 4)
    host["d2p"] = lhsT_pack(np.ascontiguousarray(g["D2"].T), 4, 8)
    host["maskneg"] = np.where(
        np.arange(128)[:, None] <= np.arange(DQ)[None, :], 0.0, NEG
    ).astype(np.float32)
    host["ones"] = np.ones((128, 1), np.float32).astype(bf)
    host["ones128"] = np.ones((128, DQ), np.float32).astype(bf)
    if affine:
        host["grep"] = np.stack([
            g[n].astype(np.float32).reshape(8, 128).T
            for n in ("g1", "b1", "g2", "b2", "g3", "b3")]).copy()

    in_maps = []
    y = g["y"].astype(np.float32)
    mem = g["mem"].astype(np.float32)
    for c in range(N_CORES):
        sl = slice(BPC * c, BPC * (c + 1))
        m = dict(host)
        # y0T [b][p, k, s]: = y[b, s, 128k+p]
        yT_ = y[sl].transpose(0, 2, 1)  # [b, D, S]
        m["y0T"] = np.ascontiguousarray(
            yT_.reshape(BPC, 8, 128, S_T).transpose(0, 2, 1, 3)).astype(bf)
        # memT [b][sm][p, 128k+sigma] = mem[b, 128sm+sigma, 128k+p]
        mm = mem[sl].reshape(BPC, 16, 128, 8, 128)  # [b, sm, sigma, k, p]
        m["memT"] = np.ascontiguousarray(
            mm.transpose(0, 1, 4, 3, 2).reshape(BPC, 16, 128, 8 * DQ)).astype(bf)
        in_maps.append(m)
    return in_maps, affine


def kernel(**inputs):
    in_maps, affine = _prep_host(inputs)
    if affine not in _CACHE:
        _CACHE[affine] = _build(affine)
    nc = _CACHE[affine]
    res = run_bass_kernel_spmd(nc, in_maps, list(range(N_CORES)))
    # outT [BPC, 128, 8, S_T] -> out[b, s, 128cc+p] = outT[b, p, cc, s]
    outs = []
    for r in res.results:
        o = r["outT"].reshape(BPC, 128, 8, S_T).transpose(0, 3, 2, 1)
        outs.append(np.ascontiguousarray(o.reshape(BPC, S_T, D)))
    return np.concatenate(outs, axis=0)


# revision 8
# speedup vs baseline: 1.1322x; 1.0042x over previous
# Trainium2 Bass kernel for nn_DecoderBlock — transposed-activation design.
#
# Sharding: data-parallel over batch — 16 elems / 8 cores = 2 per core.
#
# All activations live in SBUF in TRANSPOSED layout xT [D(part, 8 chunks), S]
# for the whole kernel; natural layout is never materialized on device (the
# host transposes the final output back, which is free).
#
# Per head h:
#   QT = Wq[h].T @ xT            (lhsT = Wq chunks, rhs = xT)      [dq, S]
#   expQT = exp((QT + mask)/sc)  -> qs = colsum via ones-matmul    [1, S]
#   softQT = expQT * bcast(1/qs) (GPSIMD partition_broadcast)
#   K/V natural per sm-tile: lhsT = xT s-chunk, rhs = Wk/Wv packed [s, 8*dq]
#   expK, V' = V / rowsum(expK); A[h] += expK[h].T @ V'[h]         [dq, dq]
#   BmT[h] = A[h].T @ softQT                                       [dq, S]
# Output (torch .view(b,w,h*d) quirk folded into a strided AP):
#   outT[cc] = sum_j WoT[j,cc].T @ BmTpack[:, w', j]   (w' = 128h+q, s = 8q+j)
# Residual + LayerNorm in transposed layout: stats over the partition axis
# via ones-matmuls, per-column scale/bias replicated with partition_broadcast.
# LFFN fully transposed: h1T = E1 @ yT, h2T = D1 @ h1T, silu, E2, D2.
import numpy as np
import ml_dtypes

import concourse.bacc as bacc
import concourse.mybir as mybir
import concourse.tile as tile
from concourse.bass_utils import run_bass_kernel_spmd

H, D, DQ, BNK, HID = 8, 1024, 128, 512, 1024
B, S_T, S_M = 16, 1024, 2048
SCALE = DQ ** 0.25
EPS = 1e-5
NEG = -200.0
N_CORES = 8
BPC = B // N_CORES

f32 = mybir.dt.float32
bf16 = mybir.dt.bfloat16
AF = mybir.ActivationFunctionType
ALU = mybir.AluOpType
bf = ml_dtypes.bfloat16


def _ln_tail(nc, sb, mean_ps, msq_ps, eps_t, scaleb, biasb):
    """Column stats [1,S] -> broadcast scale/bias into the given tiles."""
    mu = sb.tile([1, S_T], f32, tag="ln_t", bufs=4, name="mu")
    nc.scalar.activation(mu[:], mean_ps[:], AF.Identity, scale=1.0 / D)
    ex2 = sb.tile([1, S_T], f32, tag="ln_t", bufs=4, name="ex2")
    nc.scalar.activation(ex2[:], msq_ps[:], AF.Identity, scale=1.0 / D)
    var = sb.tile([1, S_T], f32, tag="ln_t", bufs=4, name="var")
    nc.vector.tensor_tensor(out=var[:], in0=mu[:], in1=mu[:], op=ALU.mult)
    nc.vector.tensor_tensor(out=var[:], in0=ex2[:], in1=var[:], op=ALU.subtract)
    sd = sb.tile([1, S_T], f32, tag="ln_t", bufs=4, name="sd")
    nc.scalar.activation(sd[:], var[:], AF.Sqrt, bias=eps_t[:])
    rstd = sb.tile([1, S_T], f32, tag="ln_r", bufs=2, name="rstd")
    nc.vector.reciprocal(rstd[:], sd[:])
    nmu = sb.tile([1, S_T], f32, tag="ln_r", bufs=2, name="nmu")
    nc.vector.scalar_tensor_tensor(
        out=nmu[:], in0=mu[:], scalar=-1.0, in1=rstd[:],
        op0=ALU.mult, op1=ALU.mult)
    nc.gpsimd.partition_broadcast(scaleb[:], rstd[:])
    nc.gpsimd.partition_broadcast(biasb[:], nmu[:])
    return scaleb, biasb


def _kv_stage(nc, tc, sb, wp, pa, yprev, memsm_dram, wk, wv, n_kv):
    """K/V natural projections + A accumulation for one batch elem.
    Returns a_sb [128, 8, DQ] bf16 (A per head). A matmuls run one sm-tile
    behind the projections so the PE never waits on the evac chain."""
    a_sb = sb.tile([128, 8, DQ], bf16, tag="a_sb", bufs=2)
    a_acc = sb.tile([128, 1024], f32, tag="a_acc", bufs=2)
    pend = []

    # NOTE: psum accumulation groups must not interleave within one bank, so
    # each sm's A-partial is a single-shot matmul set, accumulated on the DVE.
    def emit_a(sm, ek, ev):
        apart = pa.tile([128, 1024], f32, tag="apart", bufs=1)
        for h in range(H):
            nc.tensor.matmul(apart[:, DQ * h:DQ * (h + 1)],
                             ek[:, h, :], ev[:, h, :])
        if sm == 0:
            nc.vector.tensor_copy(a_acc[:], apart[:])
        else:
            nc.vector.tensor_tensor(out=a_acc[:], in0=a_acc[:], in1=apart[:],
                                    op=ALU.add)

    for sm in range(n_kv):
        if memsm_dram is None:
            def lhsT(k, sm=sm):
                return yprev[:, k, DQ * sm:DQ * (sm + 1)]
        else:
            mt = wp.tile([128, 8, DQ], bf16, tag="memsm", bufs=4)
            nc.sync.dma_start(mt[:], memsm_dram[sm])
            def lhsT(k, mt=mt):
                return mt[:, k, :]
        klo = pa.tile([128, 512], f32, tag="kv", bufs=6, name="klo")
        khi = pa.tile([128, 512], f32, tag="kv", bufs=6, name="khi")
        vlo = pa.tile([128, 512], f32, tag="kv", bufs=6, name="vlo")
        vhi = pa.tile([128, 512], f32, tag="kv", bufs=6, name="vhi")
        for k in range(8):
            lt = lhsT(k)
            nc.tensor.matmul(klo[:], lt, wk[:, k, 0:4, :], start=(k == 0), stop=(k == 7))
            nc.tensor.matmul(khi[:], lt, wk[:, k, 4:8, :], start=(k == 0), stop=(k == 7))
            nc.tensor.matmul(vlo[:], lt, wv[:, k, 0:4, :], start=(k == 0), stop=(k == 7))
            nc.tensor.matmul(vhi[:], lt, wv[:, k, 4:8, :], start=(k == 0), stop=(k == 7))
        expk = wp.tile([128, 8, DQ], bf16, tag="expk", bufs=3)
        nc.scalar.activation(expk[:, 0:4, :], klo[:], AF.Exp, scale=1.0 / SCALE)
        nc.scalar.activation(expk[:, 4:8, :], khi[:], AF.Exp, scale=1.0 / SCALE)
        krs = wp.tile([128, 8], f32, tag="krs", bufs=2)
        nc.vector.tensor_reduce(out=krs[:], in_=expk[:],
                                axis=mybir.AxisListType.X, op=ALU.add)
        krr = wp.tile([128, 8], f32, tag="krr", bufs=2)
        nc.vector.reciprocal(krr[:], krs[:])
        expv = wp.tile([128, 8, DQ], bf16, tag="expv", bufs=3)
        nc.vector.tensor_tensor(
            out=expv[:, 0:4, :], in0=vlo[:].rearrange("p (h q) -> p h q", h=4),
            in1=krr[:, 0:4].unsqueeze(2).broadcast_to([128, 4, DQ]), op=ALU.mult)
        nc.vector.tensor_tensor(
            out=expv[:, 4:8, :], in0=vhi[:].rearrange("p (h q) -> p h q", h=4),
            in1=krr[:, 4:8].unsqueeze(2).broadcast_to([128, 4, DQ]), op=ALU.mult)
        pend.append((sm, expk, expv))
        if len(pend) > 2:
            emit_a(*pend.pop(0))
    while pend:
        emit_a(*pend.pop(0))
    nc.vector.tensor_copy(a_sb[:].rearrange("p h q -> p (h q)"), a_acc[:])
    return a_sb


def _q_stage(nc, tc, sb, wp, pq, yprev, a_sb, wq, masked, maskt, ones128):
    """Q proj + softmax + BmT for one batch elem -> bmtp [128, 8, S_T].
    Pass-structured so the PE stream never waits on the softmax chain. The
    softmax denominator is computed already replicated across partitions by
    a ones[128,128] matmul, so no partition_broadcast hop is needed."""
    bmtp = sb.tile([128, 8, S_T], bf16, tag="bmtp", bufs=2)
    expqs = []
    for h in range(H):
        qt = pq.tile([128, S_T], f32, tag="qt", bufs=4)
        for k in range(8):
            nc.tensor.matmul(qt[:, 0:512], wq[:, h, k, :], yprev[:, k, 0:512],
                             start=(k == 0), stop=(k == 7))
            nc.tensor.matmul(qt[:, 512:1024], wq[:, h, k, :], yprev[:, k, 512:1024],
                             start=(k == 0), stop=(k == 7))
        if masked:
            nc.vector.tensor_tensor(out=qt[:, 0:DQ], in0=qt[:, 0:DQ],
                                    in1=maskt[:], op=ALU.add)
        expq = wp.tile([128, S_T], bf16, tag="expq", bufs=8, name=f"expq{h}")
        nc.scalar.activation(expq[:], qt[:], AF.Exp, scale=1.0 / SCALE)
        expqs.append(expq)
    for h in range(H):
        qrep = pq.tile([128, S_T], f32, tag="qt", bufs=4)
        nc.tensor.matmul(qrep[:, 0:512], ones128[:], expqs[h][:, 0:512])
        nc.tensor.matmul(qrep[:, 512:1024], ones128[:], expqs[h][:, 512:1024])
        qsb = wp.tile([128, S_T], f32, tag="qsb", bufs=2)
        nc.vector.reciprocal(qsb[:], qrep[:])
        nc.vector.tensor_tensor(out=expqs[h][:], in0=expqs[h][:], in1=qsb[:],
                                op=ALU.mult)
    for h in range(H):
        bmt = pq.tile([128, S_T], f32, tag="qt", bufs=4)
        nc.tensor.matmul(bmt[:, 0:512], a_sb[:, h, :], expqs[h][:, 0:512])
        nc.tensor.matmul(bmt[:, 512:1024], a_sb[:, h, :], expqs[h][:, 512:1024])
        nc.scalar.activation(bmtp[:, h, :], bmt[:], AF.Identity)
    return bmtp


def _out_stage(nc, tc, sb, wp, po, yprev, ynext, bmtp, wo, ones_bf, eps_t,
               rsd, scaleb, biasb, gb=None, gbi=0):
    """Wo matmul (transposed out) + residual + LN for one batch elem.
    Stats matmuls run one cc-tile behind so the PE never waits. rsd and the
    LN scale/bias tiles live in the persistent pool so this phase's pools can
    close (and the next phase start) while the normalize ops drain."""
    bmv = bmtp[:].rearrange("p h (m e) -> p (h m) e", e=8)
    mean_ps = po.tile([1, S_T], f32, tag="mean", bufs=1)
    msq_ps = po.tile([1, S_T], f32, tag="msq", bufs=1)
    pend = []

    def emit_stats(cc, rsq):
        nc.tensor.matmul(mean_ps[:, 0:512], ones_bf[:], rsd[:, cc, 0:512],
                         start=(cc == 0), stop=(cc == 7))
        nc.tensor.matmul(mean_ps[:, 512:1024], ones_bf[:], rsd[:, cc, 512:1024],
                         start=(cc == 0), stop=(cc == 7))
        nc.tensor.matmul(msq_ps[:, 0:512], ones_bf[:], rsq[:, 0:512],
                         start=(cc == 0), stop=(cc == 7))
        nc.tensor.matmul(msq_ps[:, 512:1024], ones_bf[:], rsq[:, 512:1024],
                         start=(cc == 0), stop=(cc == 7))

    for cc in range(8):
        ot = po.tile([128, S_T], f32, tag="ot", bufs=2)
        for j in range(8):
            nc.tensor.matmul(ot[:, 0:512], wo[:, j, cc, :], bmv[:, 0:512, j],
                             start=(j == 0), stop=(j == 7))
            nc.tensor.matmul(ot[:, 512:1024], wo[:, j, cc, :], bmv[:, 512:1024, j],
                             start=(j == 0), stop=(j == 7))
        nc.vector.tensor_tensor(out=rsd[:, cc, :], in0=ot[:], in1=yprev[:, cc, :],
                                op=ALU.add)
        rsq = wp.tile([128, S_T], bf16, tag="rsq", bufs=2)
        nc.scalar.activation(rsq[:], rsd[:, cc, :], AF.Square)
        pend.append((cc, rsq))
        if len(pend) > 1:
            emit_stats(*pend.pop(0))
    emit_stats(*pend.pop(0))
    _ln_tail(nc, wp, mean_ps, msq_ps, eps_t, scaleb, biasb)
    for cc in range(8):
        nc.vector.tensor_tensor(out=ynext[:, cc, :], in0=rsd[:, cc, :],
                                in1=scaleb[:], op=ALU.mult)
        nc.vector.tensor_tensor(out=ynext[:, cc, :], in0=ynext[:, cc, :],
                                in1=biasb[:], op=ALU.add)
        if gb is not None:
            nc.scalar.activation(ynext[:, cc, :], ynext[:, cc, :], AF.Identity,
                                 scale=gb[gbi][:, cc:cc + 1],
                                 bias=gb[gbi + 1][:, cc:cc + 1])


def _lffn_mm(nc, tc, sb, pf, src, dst, wtile, kn, tn, act):
    """dst[:, t, :] = act(sum_k wtile[:,k,t,:].T @ src[:, k, :]) for t<tn."""
    for t in range(tn):
        ps = pf.tile([128, S_T], f32, tag="facc", bufs=3)
        for k in range(kn):
            nc.tensor.matmul(ps[:, 0:512], wtile[:, k, t, :], src[:, k, 0:512],
                             start=(k == 0), stop=(k == kn - 1))
            nc.tensor.matmul(ps[:, 512:1024], wtile[:, k, t, :], src[:, k, 512:1024],
                             start=(k == 0), stop=(k == kn - 1))
        nc.scalar.activation(dst[:, t, :], ps[:], act)


def _build(affine: bool):
    nc = bacc.Bacc("TRN2", target_bir_lowering=False, debug=False,
                   enable_asserts=True, num_devices=N_CORES)

    def din(name, shape, dt=bf16):
        return nc.dram_tensor(name, list(shape), dt, kind="ExternalInput").ap()

    y0T_d = din("y0T", [BPC, 128, 8, S_T])
    memT_d = din("memT", [BPC, 16, 128, 8 * DQ])
    wq1_d = din("wq1", [128, 8, 8, DQ])
    wkv1_d = din("wkv1", [2, 128, 8, 8, DQ])
    wo1_d = din("wo1", [128, 8, 8, DQ])
    wq2_d = din("wq2", [128, 8, 8, DQ])
    wkv2_d = din("wkv2", [2, 128, 8, 8, DQ])
    wo2_d = din("wo2", [128, 8, 8, DQ])
    e1p_d = din("e1p", [128, 8, 4, DQ])
    d1p_d = din("d1p", [128, 4, 8, DQ])
    e2p_d = din("e2p", [128, 8, 4, DQ])
    d2p_d = din("d2p", [128, 4, 8, DQ])
    mask_d = din("maskneg", [128, DQ], f32)
    ones_d = din("ones", [128, 1])
    ones128_d = din("ones128", [128, DQ])
    if affine:
        grep_d = din("grep", [6, 128, 8], f32)

    outT = nc.dram_tensor("outT", [BPC, 128, 8, S_T], f32,
                          kind="ExternalOutput").ap()

    with tile.TileContext(nc) as tc:
        with tc.tile_pool(name="glob", bufs=1) as gl:
            maskt = gl.tile([128, DQ], f32, tag="maskt")
            nc.sync.dma_start(maskt[:], mask_d[:])
            ones_bf = gl.tile([128, 1], bf16, tag="ones")
            nc.sync.dma_start(ones_bf[:], ones_d[:])
            ones128 = gl.tile([128, DQ], bf16, tag="ones128")
            nc.sync.dma_start(ones128[:], ones128_d[:])
            eps_t = gl.tile([1, 1], f32, tag="eps")
            nc.vector.memset(eps_t[:], EPS)
            gb = None
            if affine:
                gb = [gl.tile([128, 8], f32, tag=f"gb{i}", name=f"gb{i}")
                      for i in range(6)]
                for i in range(6):
                    nc.sync.dma_start(gb[i][:], grep_d[i])

            # rotating activation generations per batch elem (bufs=2)
            def ytile(b, g):
                return gl.tile([128, 8, S_T], bf16, tag=f"yT{b}", bufs=2,
                               name=f"yT{b}_{g}")

            y0 = [ytile(b, 0) for b in range(BPC)]
            nc.sync.dma_start(y0[0][:], y0T_d[0])

            # ---- attention phases ----
            # wq loads at phase start (overlaps KV stage); wo loads at Q-stage
            # start (overlaps Q); wk/wv freed before the Q stage runs.
            def attn(yprev, ynext, memsm, wq_d, wkv_d, wo_d, masked, n_kv, gbi,
                     prefetch=()):
                with tc.tile_pool(name="work", bufs=1) as sb, \
                     tc.tile_pool(name="wqp", bufs=1) as wqp:
                    wq = wqp.tile([128, 8, 8, DQ], bf16, tag="wq")
                    nc.sync.dma_start(wq[:], wq_d[:])
                    for dst, srcd in prefetch:
                        nc.sync.dma_start(dst[:], srcd)
                    with tc.tile_pool(name="wkvp", bufs=1) as wkvp:
                        # per-chunk loads: the first K/V matmul only waits for
                        # its own k-chunk, not the whole 4MB of weights
                        wk = wkvp.tile([128, 8, 8, DQ], bf16, tag="wk")
                        wv = wkvp.tile([128, 8, 8, DQ], bf16, tag="wv")
                        for k in range(8):
                            nc.sync.dma_start(wk[:, k, :, :], wkv_d[0][:, k])
                            nc.sync.dma_start(wv[:, k, :, :], wkv_d[1][:, k])
                        with tc.tile_pool(name="kvt", bufs=1) as kvt:
                            with tc.tile_pool(name="ps_kv", bufs=1, space="PSUM") as pa:
                                a_sbs = [
                                    _kv_stage(nc, tc, sb, kvt, pa, yprev[b],
                                              None if memsm is None else memsm[b],
                                              wk, wv, n_kv)
                                    for b in range(BPC)]
                    with tc.tile_pool(name="wop", bufs=1) as wop:
                        wo = wop.tile([128, 8, 8, DQ], bf16, tag="wo")
                        nc.sync.dma_start(wo[:], wo_d[:])
                        with tc.tile_pool(name="qtp", bufs=1) as qtp:
                            with tc.tile_pool(name="ps_q", bufs=1, space="PSUM") as pq:
                                bmtps = [
                                    _q_stage(nc, tc, sb, qtp, pq, yprev[b], a_sbs[b],
                                             wq, masked, maskt, ones128)
                                    for b in range(BPC)]
                        with tc.tile_pool(name="otp", bufs=1) as otp:
                            with tc.tile_pool(name="ps_o", bufs=1, space="PSUM") as po:
                                for b in range(BPC):
                                    rsd = gl.tile([128, 8, S_T], bf16,
                                                  tag="rsd", bufs=1, name="rsd")
                                    scb = gl.tile([128, S_T], f32,
                                                  tag="ln_scaleb", bufs=1, name="scb")
                                    bib = gl.tile([128, S_T], f32,
                                                  tag="ln_biasb", bufs=1, name="bib")
                                    _out_stage(nc, tc, sb, otp, po, yprev[b], ynext[b],
                                               bmtps[b], wo, ones_bf, eps_t,
                                               rsd, scb, bib, gb=gb, gbi=gbi)

            y1 = [ytile(b, 1) for b in range(BPC)]
            attn(y0, y1, None, wq1_d, wkv1_d, wo1_d, True, 8, 0,
                 prefetch=[(y0[b], y0T_d[b]) for b in range(1, BPC)])
            y2 = [ytile(b, 2) for b in range(BPC)]
            attn(y1, y2, memT_d, wq2_d, wkv2_d, wo2_d, False, 16, 2)

            # ---- LFFN ----
            with tc.tile_pool(name="workf", bufs=1) as sb:
                with tc.tile_pool(name="wf", bufs=1) as wp:
                    e1p = wp.tile([128, 8, 4, DQ], bf16, tag="e1p")
                    nc.sync.dma_start(e1p[:], e1p_d[:])
                    d1p = wp.tile([128, 4, 8, DQ], bf16, tag="d1p")
                    nc.sync.dma_start(d1p[:], d1p_d[:])
                    e2p = wp.tile([128, 8, 4, DQ], bf16, tag="e2p")
                    nc.sync.dma_start(e2p[:], e2p_d[:])
                    d2p = wp.tile([128, 4, 8, DQ], bf16, tag="d2p")
                    nc.sync.dma_start(d2p[:], d2p_d[:])
                    for b in range(BPC):
                        h1T = sb.tile([128, 4, S_T], bf16, tag="h1T", bufs=1)
                        swT = sb.tile([128, 8, S_T], bf16, tag="swT", bufs=1)
                        g1T = sb.tile([128, 4, S_T], bf16, tag="g1T", bufs=1)
                        with tc.tile_pool(name="ps_f", bufs=1, space="PSUM") as pf:
                            _lffn_mm(nc, tc, sb, pf, y2[b], h1T, e1p, 8, 4, AF.Identity)
                            _lffn_mm(nc, tc, sb, pf, h1T, swT, d1p, 4, 8, AF.Silu)
                            _lffn_mm(nc, tc, sb, pf, swT, g1T, e2p, 8, 4, AF.Identity)
                        with tc.tile_pool(name="ps_f2", bufs=1, space="PSUM") as po:
                            # D2 matmul + residual + LN3 (writes outf f32)
                            rsd = gl.tile([128, 8, S_T], bf16, tag="rsd",
                                          bufs=1, name="rsd")
                            scb = gl.tile([128, S_T], f32, tag="ln_scaleb",
                                          bufs=1, name="scb")
                            bib = gl.tile([128, S_T], f32, tag="ln_biasb",
                                          bufs=1, name="bib")
                            mean_ps = po.tile([1, S_T], f32, tag="mean", bufs=1)
                            msq_ps = po.tile([1, S_T], f32, tag="msq", bufs=1)
                            def emit_stats(cc, rsq):
                                nc.tensor.matmul(mean_ps[:, 0:512], ones_bf[:],
                                                 rsd[:, cc, 0:512],
                                                 start=(cc == 0), stop=(cc == 7))
                                nc.tensor.matmul(mean_ps[:, 512:1024], ones_bf[:],
                                                 rsd[:, cc, 512:1024],
                                                 start=(cc == 0), stop=(cc == 7))
                                nc.tensor.matmul(msq_ps[:, 0:512], ones_bf[:],
                                                 rsq[:, 0:512],
                                                 start=(cc == 0), stop=(cc == 7))
                                nc.tensor.matmul(msq_ps[:, 512:1024], ones_bf[:],
                                                 rsq[:, 512:1024],
                                                 start=(cc == 0), stop=(cc == 7))
                            pend = []
                            for cc in range(8):
                                ps = po.tile([128, S_T], f32, tag="ot", bufs=2)
                                for k in range(4):
                                    nc.tensor.matmul(
                                        ps[:, 0:512], d2p[:, k, cc, :],
                                        g1T[:, k, 0:512],
                                        start=(k == 0), stop=(k == 3))
                                    nc.tensor.matmul(
                                        ps[:, 512:1024], d2p[:, k, cc, :],
                                        g1T[:, k, 512:1024],
                                        start=(k == 0), stop=(k == 3))
                                nc.vector.tensor_tensor(
                                    out=rsd[:, cc, :], in0=ps[:],
                                    in1=y2[b][:, cc, :], op=ALU.add)
                                rsq = sb.tile([128, S_T], bf16, tag="rsq", bufs=2)
                                nc.scalar.activation(rsq[:], rsd[:, cc, :], AF.Square)
                                pend.append((cc, rsq))
                                if len(pend) > 1:
                                    emit_stats(*pend.pop(0))
                            emit_stats(*pend.pop(0))
                            scaleb, biasb = _ln_tail(nc, sb, mean_ps, msq_ps,
                                                     eps_t, scb, bib)
                            for cc in range(8):
                                outf = sb.tile([128, S_T], f32, tag="outf", bufs=2)
                                nc.vector.tensor_tensor(
                                    out=outf[:], in0=rsd[:, cc, :],
                                    in1=scaleb[:], op=ALU.mult)
                                nc.vector.tensor_tensor(
                                    out=outf[:], in0=outf[:],
                                    in1=biasb[:], op=ALU.add)
                                if gb is not None:
                                    nc.scalar.activation(
                                        outf[:], outf[:], AF.Identity,
                                        scale=gb[4][:, cc:cc + 1],
                                        bias=gb[5][:, cc:cc + 1])
                                nc.sync.dma_start(outT[b][:, cc, :], outf[:])

    nc.compile()
    return nc


_CACHE = {}


def _prep_host(inputs):
    g = {k: np.asarray(v) for k, v in inputs.items()}
    affine = not (
        np.all(g["g1"] == 1) and np.all(g["g2"] == 1) and np.all(g["g3"] == 1)
        and np.all(g["b1"] == 0) and np.all(g["b2"] == 0) and np.all(g["b3"] == 0))

    def wq_pack(w):  # stationary: [128, h, k, dq]
        return np.ascontiguousarray(
            w.transpose(1, 0, 2).reshape(8, 128, 8, DQ).transpose(1, 2, 0, 3)
        ).astype(bf)

    def wkv_pack(wk_, wv_):  # moving: [2, 128, k, h, dq]
        def one(w):
            # w [H, D, DQ]: arr[p, k, h, :] = w[h, 128k+p, :]
            return w.transpose(1, 0, 2).reshape(8, 128, 8, DQ).transpose(1, 0, 2, 3)
        return np.ascontiguousarray(np.stack([one(wk_), one(wv_)])).astype(bf)

    def lhsT_pack(wT, kn, tn):  # [128, k, t, 128] from wT [kn*128, tn*128]
        return np.ascontiguousarray(
            wT.reshape(kn, 128, tn, DQ).transpose(1, 0, 2, 3)).astype(bf)

    host = {}
    host["wq1"] = wq_pack(g["Wq1"])
    host["wkv1"] = wkv_pack(g["Wk1"], g["Wv1"])
    host["wo1"] = lhsT_pack(np.ascontiguousarray(g["Wo1"].T), 8, 8)
    host["wq2"] = wq_pack(g["Wq2"])
    host["wkv2"] = wkv_pack(g["Wk2"], g["Wv2"])
    host["wo2"] = lhsT_pack(np.ascontiguousarray(g["Wo2"].T), 8, 8)
    host["e1p"] = lhsT_pack(np.ascontiguousarray(g["E1"].T), 8, 4)
    host["d1p"] = lhsT_pack(np.ascontiguousarray(g["D1"].T), 4, 8)
    host["e2p"] = lhsT_pack(np.ascontiguousarray(g["E2"].T), 8, 4)
    host["d2p"] = lhsT_pack(np.ascontiguousarray(g["D2"].T), 4, 8)
    host["maskneg"] = np.where(
        np.arange(128)[:, None] <= np.arange(DQ)[None, :], 0.0, NEG
    ).astype(np.float32)
    host["ones"] = np.ones((128, 1), np.float32).astype(bf)
    host["ones128"] = np.ones((128, DQ), np.float32).astype(bf)
    if affine:
        host["grep"] = np.stack([
            g[n].astype(np.float32).reshape(8, 128).T
            for n in ("g1", "b1", "g2", "b2", "g3", "b3")]).copy()

    in_maps = []
    y = g["y"].astype(np.float32)
    mem = g["mem"].astype(np.float32)
    for c in range(N_CORES):
        sl = slice(BPC * c, BPC * (c + 1))
        m = dict(host)
        # y0T [b][p, k, s]: = y[b, s, 128k+p]
        yT_ = y[sl].transpose(0, 2, 1)  # [b, D, S]
        m["y0T"] = np.ascontiguousarray(
            yT_.reshape(BPC, 8, 128, S_T).transpose(0, 2, 1, 3)).astype(bf)
        # memT [b][sm][p, 128k+sigma] = mem[b, 128sm+sigma, 128k+p]
        mm = mem[sl].reshape(BPC, 16, 128, 8, 128)  # [b, sm, sigma, k, p]
        m["memT"] = np.ascontiguousarray(
            mm.transpose(0, 1, 4, 3, 2).reshape(BPC, 16, 128, 8 * DQ)).astype(bf)
        in_maps.append(m)
    return in_maps, affine


def kernel(**inputs):
    in_maps, affine = _prep_host(inputs)
    if affine not in _CACHE:
        _CACHE[affine] = _build(affine)
    nc = _CACHE[affine]
    res = run_bass_kernel_spmd(nc, in_maps, list(range(N_CORES)))
    # outT [BPC, 128, 8, S_T] -> out[b, s, 128cc+p] = outT[b, p, cc, s]
    outs = []
    for r in res.results:
        o = r["outT"].reshape(BPC, 128, 8, S_T).transpose(0, 3, 2, 1)
        outs.append(np.ascontiguousarray(o.reshape(BPC, S_T, D)))
    return np.concatenate(outs, axis=0)
